# revision 29
# baseline (speedup 1.0000x reference)
"""Trainium2 Bass kernel for the LSTM+dense+softmax model.

Model (see reference): x[T=512, B=256, IN=256] -> LSTM(H=128) last hidden
-> dense(OUT=1000) -> softmax. Data-parallel over batch across 8 cores
(32 batch elements per core), weights replicated.

Layout: recurrent state is kept transposed [H=128 partitions, batch] so the
per-step W_hh matmuls, gate nonlinearities and cell update all run at full
partition width with no transposes. Gate pre-activations for 4 consecutive
steps share one PSUM bank: W_ih*x contributions (+bias) are accumulated
ahead of time, W_hh*h is added when the step arrives, and ScalarE applies
sigmoid/tanh directly out of PSUM.

Both matmul paths run in bfloat16 (fp32 matmuls cost 4 PE cycles/row and
are emitted as two half-speed passes; bf16 costs 1): measured 2.0x faster
end to end at rel_err ~1e-3 vs the fp32 reference. The recurrence is
latency-bound (~2.1us/step serial chain: 4x W_hh matmul -> sigmoid ->
3 DVE ops -> tanh -> h-mul); alternative schedules (finer xproj slicing,
wait-on-matmul-instead-of-ldweights, semaphore-throttled lookahead) all
measured slower on hardware.
"""

import numpy as np

import concourse.bacc as bacc
import concourse.mybir as mybir
import concourse.tile as tile
from concourse.bass_utils import run_bass_kernel_spmd

SEQ = 512
B = 256
IN = 256
H = 128
OUT = 1000
N_CORES = 8
BC = B // N_CORES  # 32 batch per core
KT = IN // H  # 2 k-tiles for the input projection
G4 = 4  # gate slot order: (i,f,g,o) in SCAN mode, (i,f,o,g) otherwise
SPB = 4  # steps per PSUM bank group (4*4*32 fp32 = one 2KB bank)
AHEAD = 4  # bank groups of x-projection lookahead
CH = 8  # timesteps per streamed x chunk (small first chunks cut startup)

F32 = mybir.dt.float32
BF16 = mybir.dt.bfloat16

import os as _os
H_BF16 = _os.environ.get("LSTM_H_BF16", "1") == "1"  # W_hh*h path in bf16
X_BF16 = _os.environ.get("LSTM_X_BF16", "1") == "1"  # W_ih*x (+bias) path in bf16
REP = int(_os.environ.get("LSTM_REP", "1"))  # timing amplification (bench only)
FUSED = _os.environ.get("LSTM_FUSED", "1") == "1"  # custom-DVE fused cell update

_CACHE = {}


# --- custom DVE op: fused LSTM cell products ------------------------------
# One instruction computing, over [P, 2, N] paged operands,
#   page 0: out = i * (2*sg - 1)   (= i * tanh(g~), sg = sigmoid(2*g~))
#   page 1: out = f * c
# replacing the stock MULTIPLY + MULTIPLY,SUBTRACT pair on the recurrence's
# serial critical path. Registered into concourse.dve_ops.OPS at import time
# (the documented per-NEFF extension point; the uop table is regenerated for
# every compile, no firmware change involved).
def _lstm_cell_ref(in0, in1, s0, s1, imm2):
    a = in0.reshape((in0.shape[0], 2, -1)).astype(np.float32)
    b = in1.reshape(a.shape).astype(np.float32)
    out = np.empty_like(a)
    out[:, 0, :] = b[:, 0, :] * (2.0 * a[:, 0, :] - 1.0)
    out[:, 1, :] = b[:, 1, :] * a[:, 1, :]
    return out.reshape(in0.shape)


def _register_lstm_cell_op():
    import re
    import concourse.dve_ops as dve_ops_mod
    from concourse.dve_ops import OPS, CUSTOM_DVE_SPECS, DveOp
    from concourse.dve_spec import Spec, Src0, Src1, One, SubIdx

    name = "LSTM_CELL_ANT"
    for op in OPS:
        if op.name == name:
            return op
    pg0 = SubIdx < One  # 1.0 on page 0, 0.0 on page 1
    spec = Spec(
        body=Src1 * (Src0 * (One + pg0) - pg0),
        reference=_lstm_cell_ref,
    )
    probe = DveOp(name, spec, subdim=True, uops_sha={})
    OPS.append(probe)
    dve_ops_mod._SUB_OPCODE_FOR_NAME[name] = (
        dve_ops_mod._CUSTOM_DVE_ROW_BASE + len(OPS) - 1
    )
    CUSTOM_DVE_SPECS[name] = spec
    shas = {}
    for ver in ("v3", "v4"):
        try:
            probe.compile(ver)
            shas[ver] = probe.uops_sha.get(ver, "")
        except ValueError as e:
            m = re.search(r'"([0-9a-f]{16})"', str(e))
            if not m:
                raise
            shas[ver] = m.group(1)
    final = DveOp(name, spec, subdim=True, uops_sha=shas)
    OPS[-1] = final
    from concourse.dve_table_gen import free_opcode_rows

    row = dve_ops_mod.get_dve_sub_opcode(name)
    assert row in free_opcode_rows("TRN2"), (name, row)
    return final


LSTM_CELL_OP = _register_lstm_cell_op() if FUSED else None

SCAN = _os.environ.get("LSTM_SCAN", "1") == "1"  # segmented-scan cell update
SIG3 = _os.environ.get("LSTM_SIG3", "1") == "1"  # split sigmoid: (i,f,g) + o


# --- custom DVE op: full cell update in ONE instruction -------------------
# Streams interleaved pairs (even, odd) = ((sg_b, c_b) via in0, (i_b, f_b)
# via in1) and emits at each odd position  c_new_b = i_b*(2*sg_b-1) + f_b*c_b.
# The stock DSL has no per-pair scan reset, so the uop program is hand-built:
# a 3-state ping-pong FSM (even resets the running sum to i*(2sg-1), odd adds
# f*c), derived by patching the lowered template of a plain scan spec.
def _lstm_scan_ref(in0, in1, s0, s1, imm2):
    P = in0.shape[0]
    a = in0.reshape(P, -1, 2).astype(np.float32)  # sg, c
    b = np.asarray(in1).reshape(P, -1, 2).astype(np.float32)  # i, f
    c0 = s0 if isinstance(s0, float) else s0.reshape(P, 1)
    it = b[:, :, 0] * (c0 * a[:, :, 0] - 1.0)
    fc = b[:, :, 1] * a[:, :, 1]
    out = np.empty_like(a)
    out[:, :, 0] = it
    out[:, :, 1] = it + fc
    return out.reshape(in0.shape)


def _register_lstm_scan_op():
    import copy
    import concourse.dve_ops as dve_ops_mod
    from concourse.dve_ops import OPS, CUSTOM_DVE_SPECS, DveOp
    from concourse.dve_spec import Spec, Src0, Src1, One, C0, scan, lower, AluOp
    from concourse.dve_uop import DveOpSpec, Trigger
    from dataclasses import dataclass

    name = "LSTM_SCAN_ANT"
    for op in OPS:
        if op.name == name:
            return op

    # template: x = (Src0*C0 - 1)*Src1 ; S = scan(ADD, x)
    x = ((Src0 * C0) - One) * Src1
    spec = Spec(body=scan(AluOp.ADD, x), reference=_lstm_scan_ref)

    @dataclass(frozen=True)
    class _PatchedDveOp(DveOp):
        programs: dict = None

        def compile(self, ver):
            return self.programs[ver]

    programs = {}
    shas = {}
    OPS.append(None)  # reserve the row before computing opcode
    row_idx = dve_ops_mod._CUSTOM_DVE_ROW_BASE + len(OPS) - 1
    dve_ops_mod._SUB_OPCODE_FOR_NAME[name] = row_idx
    CUSTOM_DVE_SPECS[name] = spec
    for ver in ("v3", "v4"):
        uops = lower(spec, ver=ver)
        assert len(uops) == 2, len(uops)
        steady = uops[1]
        # identify blocks: blk_mul2 (MUL Src0*C0), blk_sub (-1), blk_scan (ADD CURR)
        blk_mul2 = blk_sub = blk_scan = None
        for i, dp in enumerate(steady.datapath_config):
            if dp.op == AluOp.MULTIPLY and blk_mul2 is None:
                blk_mul2 = i
            elif dp.op == AluOp.SUBTRACT and blk_sub is None:
                blk_sub = i
            elif dp.op == AluOp.ADD and "CURR" in dp.alu_src0.name:
                blk_scan = i
        assert None not in (blk_mul2, blk_sub, blk_scan), (
            blk_mul2, blk_sub, blk_scan)

        def mk_even(nxt):
            u = copy.deepcopy(steady)
            dp = u.datapath_config[blk_scan]
            dp.op = AluOp.BYPASS
            dp.alu_src0 = dp.alu_src1  # pass x through; resets running sum
            u.trigger = (Trigger.SRC_TENSOR_DONE, Trigger.COUNT, Trigger.NONE)
            u.next_uop = (0, nxt, 0)
            u.repeat_count = 1
            return u

        def mk_odd(nxt):
            u = copy.deepcopy(steady)
            dpm = u.datapath_config[blk_mul2]
            dpm.op = AluOp.BYPASS  # x1 = Src0 (skip *C0)
            dps = u.datapath_config[blk_sub]
            dps.op = AluOp.BYPASS
            dps.alu_src1 = dps.alu_src0  # x2 = x1 (skip -1)
            u.trigger = (Trigger.SRC_TENSOR_DONE, Trigger.COUNT, Trigger.NONE)
            u.next_uop = (0, nxt, 0)
            u.repeat_count = 1
            return u

        prog = [mk_even(1), mk_odd(2), mk_even(1)]
        dos = DveOpSpec(name=name, opcode=row_idx, uops=prog, rd1_en=True)
        programs[ver] = dos
        shas[ver] = dos.sha(ver)

    final = _PatchedDveOp(name, spec, subdim=False, uops_sha=shas,
                          programs=programs)
    OPS[-1] = final
    from concourse.dve_table_gen import free_opcode_rows

    assert row_idx in free_opcode_rows("TRN2"), (name, row_idx)
    return final


LSTM_SCAN_OP = _register_lstm_scan_op() if SCAN else None

# torch gate block (i,f,g,o) -> our slot; GPRE = slot of the pre-doubled g
if SCAN:
    PERM, GPRE = [0, 1, 2, 3], 2
else:
    PERM, GPRE = [0, 1, 3, 2], 3


def _build(T):
    ngrp = T // SPB
    ch = min(CH, T)
    HD = BF16 if H_BF16 else F32
    XD = BF16 if X_BF16 else F32
    nc = bacc.Bacc("TRN2", target_bir_lowering=False, debug=False)

    xT = nc.declare_dram_parameter("xT", [H, KT, T, BC], XD, isOutput=False)
    whhT = nc.declare_dram_parameter("whhT", [H, G4, H], HD, isOutput=False)
    wihT = nc.declare_dram_parameter("wihT", [H, KT, G4, H], XD, isOutput=False)
    bias4 = nc.declare_dram_parameter("bias4", [G4, H], XD, isOutput=False)
    ind4 = nc.declare_dram_parameter("ind4", [G4, SPB * G4 * BC], XD, isOutput=False)
    wdT = nc.declare_dram_parameter("wdT", [H, OUT], F32, isOutput=False)
    bd = nc.declare_dram_parameter("bd", [1, OUT], F32, isOutput=False)
    out = nc.declare_dram_parameter("out", [BC, OUT], F32, isOutput=True)

    NSPLIT = 512  # dense tail: first PSUM bank columns
    NREST = OUT - NSPLIT

    with tile.TileContext(nc) as tc:
        with (
            tc.tile_pool(name="const", bufs=1) as constp,
            tc.tile_pool(name="xs", bufs=6) as xpool,
            tc.tile_pool(name="state", bufs=1) as state,
            tc.tile_pool(name="work", bufs=3) as work,
        ):
            whh_s = constp.tile([H, G4, H], HD)
            wih_s = constp.tile([H, KT, G4, H], XD)
            bias_s = constp.tile([G4, H], XD)
            ind_s = constp.tile([G4, SPB * G4 * BC], XD)
            wd_s = constp.tile([H, OUT], F32)
            bd_s = constp.tile([1, OUT], F32)
            ones1 = constp.tile([1, BC], F32)
            nc.gpsimd.dma_start(whh_s[:], whhT[:])
            nc.gpsimd.dma_start(wih_s[:], wihT[:])
            nc.gpsimd.dma_start(bias_s[:], bias4[:])
            nc.gpsimd.dma_start(ind_s[:], ind4[:])
            nc.gpsimd.dma_start(wd_s[:], wdT[:])
            nc.gpsimd.dma_start(bd_s[:], bd[:])
            nc.vector.memset(ones1[:], 1.0)

            # persistent state: h transposed [H, BC].
            # W = [sig(i) sig(f) sig(o) sig(2g) | c]: the sigmoid of all 4
            # (pre-scaled) gates lands in W[:,0:128] right next to the cell
            # state c in W[:,128:160], so [i|f] (x) [sig2g|c] is one
            # contiguous 64-wide multiply. tanh(g) = 2*sig(2g)-1 is folded
            # into the cell update (g weights are pre-doubled on the host).
            # (A 2-op cell update via a duplicated-sigmoid scatter measured
            # dead even on HW: the saved DVE op's fixed cost reappears in
            # the 2x-wider activation. This layout is the local optimum.)
            hT = state.tile([H, BC], HD)
            hT32 = state.tile([H, BC], F32)
            # SCAN mode: two ping-pong cell sets, each 4 blocks of 2*BC
            # (i, f, sg/c, o) with gate values at even offsets and the cell
            # state c at the odd offsets of the sg block. Otherwise the flat
            # [i f o sg | c] layout.
            W = state.tile([H, 16 * BC] if SCAN else [H, 5 * BC], F32)

            nchunk = (T + ch - 1) // ch
            xtiles = [None] * nchunk

            def ensure_chunk(ci):
                if xtiles[ci] is None:
                    xt = xpool.tile([H, KT, ch, BC], XD)
                    nc.gpsimd.dma_start(
                        xt[:], xT[:, :, ci * ch : (ci + 1) * ch, :]
                    )
                    xtiles[ci] = xt

            for _rep in range(REP):
              if True:
                xtiles = [None] * nchunk
                nc.vector.memset(hT[:], 0.0)
                nc.vector.memset(W[:], 0.0)
                psump_cm = tc.tile_pool(name=f"psum{_rep}", bufs=AHEAD + 2, space="PSUM")
                psump = psump_cm.__enter__()
                pstiles = [None] * ngrp

                from concourse.tile import add_dep_helper

                def alloc_group(g, dep=None):
                    # allocate the PSUM bank for group g and seed it with the
                    # bias: the ONE start=True matmul covering the whole bank
                    # (start=True clears has_written bank-wide, so it must be
                    # the single first writer; everything after accumulates)
                    t0 = g * SPB
                    ensure_chunk(t0 // ch)
                    ps = psump.tile([H, SPB, G4, BC], F32)
                    pstiles[g] = ps
                    mm = nc.tensor.matmul(
                        ps[:].rearrange("p a g b -> p (a g b)"),
                        bias_s[:],
                        ind_s[:],
                        start=True,
                        stop=False,
                        skip_group_check=True,
                    )
                    if dep is not None:
                        add_dep_helper(mm.ins, dep.ins, reason="xproj window anchor")

                def emit_xproj_mms(g, pairs, dep=None):
                    # accumulate W_ih*x contributions (gi, kt) for group g
                    t0 = g * SPB
                    ci = t0 // ch
                    xt = xtiles[ci]
                    s0 = t0 - ci * ch
                    ps = pstiles[g]
                    for gi, kt in pairs:
                        mm = nc.tensor.matmul(
                            ps[:, :, gi, :],
                            wih_s[:, kt, gi, :],
                            xt[:, kt, s0 : s0 + SPB, :],
                            start=False,
                            stop=False,
                            skip_group_check=True,
                        )
                        if dep is not None:
                            add_dep_helper(mm.ins, dep.ins, reason="xproj window anchor")

                ALL_PAIRS = [(gi, kt) for gi in range(G4) for kt in range(KT)]

                for g in range(min(AHEAD, ngrp)):
                    alloc_group(g)
                    emit_xproj_mms(g, ALL_PAIRS)

                # next group's xproj is spread across this group's four
                # steps, each sub-batch dep-anchored on that step's sigmoid
                # so the scheduler places it in the step's PE-idle window
                # (one 9-op batch at a group boundary overflows the window)
                XPIECES = [[], ALL_PAIRS[0:3], ALL_PAIRS[3:6], ALL_PAIRS[6:8]]
                for g in range(ngrp):
                    ps = pstiles[g]
                    for s in range(SPB):
                        t_glob = g * SPB + s
                        # W_hh * h into the gate bank (critical path).
                        for gi in ((0, 1, 2, 3) if SCAN else (3, 0, 1, 2)):
                            nc.tensor.matmul(
                                ps[:, s, gi, :],
                                whh_s[:, gi, :],
                                hT[:],
                                start=False,
                                stop=(gi == (3 if SCAN else 2)),
                                skip_group_check=True,
                            )
                        tct = work.tile([H, BC], F32)
                        ht_dst = hT32 if t_glob == T - 1 else hT
                        if SCAN:
                            Xb = 0 if (t_glob % 2 == 0) else 8 * BC
                            Yb = 8 * BC - Xb
                            if SIG3:
                                # sig(i,f,g) only waits on the first 3 W_hh
                                # matmuls; sig(o) runs behind it on ScalarE,
                                # hidden under the DVE cell op (o is not
                                # needed until the h-mul)
                                sig_inst = nc.scalar.activation(
                                    W[:, Xb : Xb + 6 * BC].rearrange(
                                        "p (g b two) -> p g b two", g=3, two=2
                                    )[:, :, :, 0],
                                    ps[:, s, 0:3, :],
                                    mybir.ActivationFunctionType.Sigmoid,
                                )
                                nc.scalar.activation(
                                    W[:, Xb + 6 * BC : Xb + 8 * BC].rearrange(
                                        "p (b two) -> p b two", two=2
                                    )[:, :, 0],
                                    ps[:, s, 3, :],
                                    mybir.ActivationFunctionType.Sigmoid,
                                )
                            else:
                                # sigmoid of all 4 gates -> even offsets of X
                                sig_inst = nc.scalar.activation(
                                    W[:, Xb : Xb + 8 * BC].rearrange(
                                        "p (g b two) -> p g b two", g=4, two=2
                                    )[:, :, :, 0],
                                    ps[:, s, :, :],
                                    mybir.ActivationFunctionType.Sigmoid,
                                )
                            # whole cell update in one segmented-scan DVE op:
                            # set Y's sg/c block gets [junk | c_new] pairs
                            in1 = W[:, Xb : Xb + 4 * BC].rearrange(
                                "p (j b two) -> p j b two", j=2, two=2
                            )[:, :, :, 0].rearrange("p j b -> p b j")
                            nc.vector._custom_dve(
                                LSTM_SCAN_OP,
                                out=W[:, Yb + 4 * BC : Yb + 6 * BC],
                                in0=W[:, Xb + 4 * BC : Xb + 6 * BC],
                                in1=in1,
                                s0=2.0,
                            )
                            nc.scalar.activation(
                                tct[:],
                                W[:, Yb + 4 * BC : Yb + 6 * BC].rearrange(
                                    "p (b two) -> p b two", two=2
                                )[:, :, 1],
                                mybir.ActivationFunctionType.Tanh,
                            )
                            nc.vector.tensor_mul(
                                ht_dst[:],
                                W[:, Xb + 6 * BC : Xb + 8 * BC].rearrange(
                                    "p (b two) -> p b two", two=2
                                )[:, :, 0],
                                tct[:],
                            )
                        else:
                            prod = work.tile([H, 2 * BC], F32)
                            # sigmoid of all 4 gates (g pre-scaled by 2)
                            sig_inst = nc.scalar.activation(
                                W[:, 0 : 4 * BC].rearrange("p (g b) -> p g b", g=4),
                                ps[:, s, :, :],
                                mybir.ActivationFunctionType.Sigmoid,
                            )
                            if FUSED:
                                # fused DVE op: prod = [i*(2*sig2g-1) | f*c]
                                nc.vector._custom_dve(
                                    LSTM_CELL_OP,
                                    out=prod[:].rearrange("p (s b) -> p s b", s=2),
                                    in0=W[:, 3 * BC : 5 * BC].rearrange(
                                        "p (s b) -> p s b", s=2
                                    ),
                                    in1=W[:, 0 : 2 * BC].rearrange(
                                        "p (s b) -> p s b", s=2
                                    ),
                                )
                                # c = i*tanh(g) + f*c
                                nc.vector.tensor_add(
                                    W[:, 4 * BC : 5 * BC],
                                    prod[:, 0:BC],
                                    prod[:, BC : 2 * BC],
                                )
                            else:
                                # prod = [i*sig2g | f*c]
                                nc.vector.tensor_mul(
                                    prod[:], W[:, 0 : 2 * BC], W[:, 3 * BC : 5 * BC]
                                )
                                # c = 2*prod0 - i + prod1
                                nc.vector.scalar_tensor_tensor(
                                    tct[:], prod[:, 0:BC], 2.0, W[:, 0:BC],
                                    op0=mybir.AluOpType.mult,
                                    op1=mybir.AluOpType.subtract,
                                )
                                nc.vector.tensor_add(
                                    W[:, 4 * BC : 5 * BC], tct[:],
                                    prod[:, BC : 2 * BC],
                                )
                            nc.scalar.activation(
                                tct[:],
                                W[:, 4 * BC : 5 * BC],
                                mybir.ActivationFunctionType.Tanh,
                            )
                            nc.vector.tensor_mul(
                                ht_dst[:], W[:, 2 * BC : 3 * BC], tct[:]
                            )
                        if g + AHEAD < ngrp:
                            if s == 0:
                                alloc_group(g + AHEAD, dep=sig_inst)
                            emit_xproj_mms(g + AHEAD, XPIECES[s], dep=sig_inst)
                    pstiles[g] = None

                psump_cm.__exit__(None, None, None)
            # dense + softmax tail
            with tc.tile_pool(name="psd", bufs=2, space="PSUM") as psumd:
                lA = psumd.tile([BC, NSPLIT], F32)
                lB = psumd.tile([BC, NREST], F32)
                # bias matmuls first: they don't depend on the last hidden
                # state, so they run during the final step's elementwise ops
                nc.tensor.matmul(
                    lA[:], ones1[:], bd_s[:, 0:NSPLIT], start=True, stop=False,
                    skip_group_check=True,
                )
                nc.tensor.matmul(
                    lB[:], ones1[:], bd_s[:, NSPLIT:OUT], start=True, stop=False,
                    skip_group_check=True,
                )
                nc.tensor.matmul(
                    lA[:], hT32[:], wd_s[:, 0:NSPLIT], start=False, stop=True,
                    skip_group_check=True,
                )
                nc.tensor.matmul(
                    lB[:], hT32[:], wd_s[:, NSPLIT:OUT], start=False, stop=True,
                    skip_group_check=True,
                )
                mA = work.tile([BC, 1], F32)
                mB = work.tile([BC, 1], F32)
                mneg = work.tile([BC, 1], F32)
                sA = work.tile([BC, 1], F32)
                sB = work.tile([BC, 1], F32)
                stot = work.tile([BC, 1], F32)
                rec = work.tile([BC, 1], F32)
                sm = work.tile([BC, OUT], F32)
                nc.vector.reduce_max(mA[:], lA[:], axis=mybir.AxisListType.X)
                nc.vector.reduce_max(mB[:], lB[:], axis=mybir.AxisListType.X)
                nc.vector.tensor_max(mA[:], mA[:], mB[:])
                nc.vector.tensor_scalar_mul(mneg[:], mA[:], -1.0)
                nc.scalar.activation(
                    sm[:, 0:NSPLIT], lA[:], mybir.ActivationFunctionType.Exp,
                    bias=mneg[:], accum_out=sA[:],
                )
                nc.scalar.activation(
                    sm[:, NSPLIT:OUT], lB[:], mybir.ActivationFunctionType.Exp,
                    bias=mneg[:], accum_out=sB[:],
                )
                nc.vector.tensor_add(stot[:], sA[:], sB[:])
                nc.vector.reciprocal(rec[:], stot[:])
                nc.vector.tensor_scalar_mul(sm[:], sm[:], rec[:])
                nc.gpsimd.dma_start(out[:], sm[:])

    nc.compile()
    return nc


def _get_nc(T):
    key = (T, REP, H_BF16, X_BF16, FUSED, SCAN, SIG3)
    if key not in _CACHE:
        _CACHE[key] = _build(T)
    return _CACHE[key]


def prep_inputs(x, w_ih, w_hh, b_ih, b_hh, w_dense, b_dense):
    import ml_dtypes
    xd = ml_dtypes.bfloat16 if X_BF16 else np.float32
    hd = ml_dtypes.bfloat16 if H_BF16 else np.float32
    T = x.shape[0]
    x = np.ascontiguousarray(x, dtype=np.float32)
    # xT[k, kt, t, b] = x[t, b, kt*128+k]
    xt_all = np.ascontiguousarray(
        x.reshape(T, B, KT, H).transpose(3, 2, 0, 1).astype(xd)
    )
    whhT = np.ascontiguousarray(
        w_hh.reshape(4, H, H)[PERM].transpose(2, 0, 1).astype(hd)
    )
    wihT = np.ascontiguousarray(
        w_ih.reshape(4, H, KT, H)[PERM].transpose(3, 2, 0, 1).astype(xd)
    )
    bias4 = (b_ih + b_hh).reshape(4, H)[PERM].astype(np.float32)
    # pre-scale the g gate by 2: tanh(x) = 2*sigmoid(2x) - 1
    whhT = whhT.copy(); wihT = wihT.copy()
    whhT[:, GPRE, :] = whhT[:, GPRE, :] * np.asarray(2.0, whhT.dtype)
    wihT[:, :, GPRE, :] = wihT[:, :, GPRE, :] * np.asarray(2.0, wihT.dtype)
    bias4[GPRE] *= 2.0
    bias4 = np.ascontiguousarray(bias4.astype(xd))
    # ind4[g, n] for n = s*(G4*BC) + gq*BC + b  -> 1.0 iff gq == g
    ind4 = np.zeros((G4, SPB * G4 * BC), dtype=xd)
    nidx = np.arange(SPB * G4 * BC)
    gq = (nidx // BC) % G4
    for g in range(G4):
        ind4[g, gq == g] = 1.0
    wdT = np.ascontiguousarray(w_dense.T, dtype=np.float32)
    bd = np.ascontiguousarray(b_dense.reshape(1, OUT), dtype=np.float32)

    in_maps = []
    for c in range(N_CORES):
        in_maps.append(
            {
                "xT": np.ascontiguousarray(xt_all[:, :, :, c * BC : (c + 1) * BC]),
                "whhT": whhT,
                "wihT": wihT,
                "bias4": bias4,
                "ind4": ind4,
                "wdT": wdT,
                "bd": bd,
            }
        )
    return in_maps


def kernel(x, w_ih, w_hh, b_ih, b_hh, w_dense, b_dense):
    x = np.asarray(x)
    T = x.shape[0]
    nc = _get_nc(T)
    in_maps = prep_inputs(
        np.asarray(x), np.asarray(w_ih), np.asarray(w_hh),
        np.asarray(b_ih), np.asarray(b_hh),
        np.asarray(w_dense), np.asarray(b_dense),
    )
    res = run_bass_kernel_spmd(nc, in_maps, list(range(N_CORES)))
    return np.concatenate(
        [res.results[c]["out"] for c in range(N_CORES)], axis=0
    ).astype(np.float32)



# revision 33
# speedup vs baseline: 1.1947x; 1.1947x over previous
"""Trainium2 Bass kernel for the LSTM+dense+softmax model.

Model (see reference): x[T=512, B=256, IN=256] -> LSTM(H=128) last hidden
-> dense(OUT=1000) -> softmax. Data-parallel over batch across 8 cores
(32 batch elements per core), weights replicated.

Layout: recurrent state is kept transposed [H=128 partitions, batch] so the
per-step W_hh matmuls, gate nonlinearities and cell update all run at full
partition width with no transposes. Gate pre-activations for 4 consecutive
steps share one PSUM bank: W_ih*x contributions (+bias) are accumulated
ahead of time, W_hh*h is added when the step arrives, and ScalarE applies
sigmoid directly out of PSUM. Both matmul paths run in bfloat16.

The recurrence is LATENCY-bound: 512 serial steps, each a cross-engine
dependency chain whose cost is per-instruction fixed overheads (ScalarE op
~300ns, DVE op ~170ns, PE SBUF fill ~180ns, ~35ns/semaphore hop), not
data width. Optimizations here, measured per step on hardware:
  - one 4-gate sigmoid (g pre-doubled; tanh(g)=2*sig(2g)-1 folded into the
    cell update) -- a split (i,f,g)+(o) sigmoid measured slower twice.
  - the ENTIRE cell update c=i*tanh(g)+f*c is ONE custom DVE instruction
    (LSTM_SCAN_ANT): a hand-built 3-uop ping-pong FSM over interleaved
    (even,odd) element pairs -- even elements compute i*(2sg-1) resetting
    the running sum, odd elements add f*c and emit c_new. Gate/cell values
    live interleaved in two ping-pong "sets" so every operand of every op
    stays a single affine access pattern. (The stock DSL has no segmented
    scan; the uop program is derived by patching a lowered scan template.)
  - x-projection matmuls + the bias matmul are dep-anchored (add_dep_helper
    on each step's sigmoid) so the simulation-guided scheduler spreads them
    into per-step PE-idle windows; emission-order/priority placement alone
    measured +280..560ns spikes on group-boundary steps.
  - dense-layer bias matmuls precede the h matmuls (no dependency on h),
    small x chunks (CH=8) + AHEAD=2 priming cut kernel head time.
Step ~1650ns: sig4(374) SCAN(240) tanh(321) hmul(191) 4x W_hh mm(268) +
sem hops. Measured slower/dead: sigmoid split, per-element DVE ops (3-op
cell), Pool offload, emission-order xproj placement, fp32 matmul paths.
"""

import numpy as np

import concourse.bacc as bacc
import concourse.mybir as mybir
import concourse.tile as tile
from concourse.bass_utils import run_bass_kernel_spmd

SEQ = 512
B = 256
IN = 256
H = 128
OUT = 1000
N_CORES = 8
BC = B // N_CORES  # 32 batch per core
KT = IN // H  # 2 k-tiles for the input projection
G4 = 4  # gate slot order: (i,f,g,o) in SCAN mode, (i,f,o,g) otherwise
SPB = 4  # steps per PSUM bank group (4*4*32 fp32 = one 2KB bank)
AHEAD = 2  # bank groups of x-projection lookahead
CH = 8  # timesteps per streamed x chunk (small first chunks cut startup)

F32 = mybir.dt.float32
BF16 = mybir.dt.bfloat16

import os as _os
H_BF16 = _os.environ.get("LSTM_H_BF16", "1") == "1"  # W_hh*h path in bf16
X_BF16 = _os.environ.get("LSTM_X_BF16", "1") == "1"  # W_ih*x (+bias) path in bf16
REP = int(_os.environ.get("LSTM_REP", "1"))  # timing amplification (bench only)
FUSED = _os.environ.get("LSTM_FUSED", "1") == "1"  # custom-DVE fused cell update

_CACHE = {}


# --- custom DVE op: fused LSTM cell products ------------------------------
# One instruction computing, over [P, 2, N] paged operands,
#   page 0: out = i * (2*sg - 1)   (= i * tanh(g~), sg = sigmoid(2*g~))
#   page 1: out = f * c
# replacing the stock MULTIPLY + MULTIPLY,SUBTRACT pair on the recurrence's
# serial critical path. Registered into concourse.dve_ops.OPS at import time
# (the documented per-NEFF extension point; the uop table is regenerated for
# every compile, no firmware change involved).
def _lstm_cell_ref(in0, in1, s0, s1, imm2):
    a = in0.reshape((in0.shape[0], 2, -1)).astype(np.float32)
    b = in1.reshape(a.shape).astype(np.float32)
    out = np.empty_like(a)
    out[:, 0, :] = b[:, 0, :] * (2.0 * a[:, 0, :] - 1.0)
    out[:, 1, :] = b[:, 1, :] * a[:, 1, :]
    return out.reshape(in0.shape)


def _register_lstm_cell_op():
    import re
    import concourse.dve_ops as dve_ops_mod
    from concourse.dve_ops import OPS, CUSTOM_DVE_SPECS, DveOp
    from concourse.dve_spec import Spec, Src0, Src1, One, SubIdx

    name = "LSTM_CELL_ANT"
    for op in OPS:
        if op.name == name:
            return op
    pg0 = SubIdx < One  # 1.0 on page 0, 0.0 on page 1
    spec = Spec(
        body=Src1 * (Src0 * (One + pg0) - pg0),
        reference=_lstm_cell_ref,
    )
    probe = DveOp(name, spec, subdim=True, uops_sha={})
    OPS.append(probe)
    dve_ops_mod._SUB_OPCODE_FOR_NAME[name] = (
        dve_ops_mod._CUSTOM_DVE_ROW_BASE + len(OPS) - 1
    )
    CUSTOM_DVE_SPECS[name] = spec
    shas = {}
    for ver in ("v3", "v4"):
        try:
            probe.compile(ver)
            shas[ver] = probe.uops_sha.get(ver, "")
        except ValueError as e:
            m = re.search(r'"([0-9a-f]{16})"', str(e))
            if not m:
                raise
            shas[ver] = m.group(1)
    final = DveOp(name, spec, subdim=True, uops_sha=shas)
    OPS[-1] = final
    from concourse.dve_table_gen import free_opcode_rows

    row = dve_ops_mod.get_dve_sub_opcode(name)
    assert row in free_opcode_rows("TRN2"), (name, row)
    return final


LSTM_CELL_OP = _register_lstm_cell_op() if FUSED else None

SCAN = _os.environ.get("LSTM_SCAN", "1") == "1"  # segmented-scan cell update
# split sigmoid (i,f,g)+(o): measured ~70ns/step SLOWER than one 4-gate
# sigmoid even with dep-anchored xproj — the second Act op's occupancy
# outweighs the earlier start. Keep off.
SIG3 = _os.environ.get("LSTM_SIG3", "0") == "1"


# --- custom DVE op: full cell update in ONE instruction -------------------
# Streams interleaved pairs (even, odd) = ((sg_b, c_b) via in0, (i_b, f_b)
# via in1) and emits at each odd position  c_new_b = i_b*(2*sg_b-1) + f_b*c_b.
# The stock DSL has no per-pair scan reset, so the uop program is hand-built:
# a 3-state ping-pong FSM (even resets the running sum to i*(2sg-1), odd adds
# f*c), derived by patching the lowered template of a plain scan spec.
def _lstm_scan_ref(in0, in1, s0, s1, imm2):
    P = in0.shape[0]
    a = in0.reshape(P, -1, 2).astype(np.float32)  # sg, c
    b = np.asarray(in1).reshape(P, -1, 2).astype(np.float32)  # i, f
    c0 = s0 if isinstance(s0, float) else s0.reshape(P, 1)
    it = b[:, :, 0] * (c0 * a[:, :, 0] - 1.0)
    fc = b[:, :, 1] * a[:, :, 1]
    out = np.empty_like(a)
    out[:, :, 0] = it
    out[:, :, 1] = it + fc
    return out.reshape(in0.shape)


def _register_lstm_scan_op():
    import copy
    import concourse.dve_ops as dve_ops_mod
    from concourse.dve_ops import OPS, CUSTOM_DVE_SPECS, DveOp
    from concourse.dve_spec import Spec, Src0, Src1, One, C0, scan, lower, AluOp
    from concourse.dve_uop import DveOpSpec, Trigger
    from dataclasses import dataclass

    name = "LSTM_SCAN_ANT"
    for op in OPS:
        if op.name == name:
            return op

    # template: x = (Src0*C0 - 1)*Src1 ; S = scan(ADD, x)
    x = ((Src0 * C0) - One) * Src1
    spec = Spec(body=scan(AluOp.ADD, x), reference=_lstm_scan_ref)

    @dataclass(frozen=True)
    class _PatchedDveOp(DveOp):
        programs: dict = None

        def compile(self, ver):
            return self.programs[ver]

    programs = {}
    shas = {}
    OPS.append(None)  # reserve the row before computing opcode
    row_idx = dve_ops_mod._CUSTOM_DVE_ROW_BASE + len(OPS) - 1
    dve_ops_mod._SUB_OPCODE_FOR_NAME[name] = row_idx
    CUSTOM_DVE_SPECS[name] = spec
    for ver in ("v3", "v4"):
        uops = lower(spec, ver=ver)
        assert len(uops) == 2, len(uops)
        steady = uops[1]
        # identify blocks: blk_mul2 (MUL Src0*C0), blk_sub (-1), blk_scan (ADD CURR)
        blk_mul2 = blk_sub = blk_scan = None
        for i, dp in enumerate(steady.datapath_config):
            if dp.op == AluOp.MULTIPLY and blk_mul2 is None:
                blk_mul2 = i
            elif dp.op == AluOp.SUBTRACT and blk_sub is None:
                blk_sub = i
            elif dp.op == AluOp.ADD and "CURR" in dp.alu_src0.name:
                blk_scan = i
        assert None not in (blk_mul2, blk_sub, blk_scan), (
            blk_mul2, blk_sub, blk_scan)

        def mk_even(nxt):
            u = copy.deepcopy(steady)
            dp = u.datapath_config[blk_scan]
            dp.op = AluOp.BYPASS
            dp.alu_src0 = dp.alu_src1  # pass x through; resets running sum
            u.trigger = (Trigger.SRC_TENSOR_DONE, Trigger.COUNT, Trigger.NONE)
            u.next_uop = (0, nxt, 0)
            u.repeat_count = 1
            return u

        def mk_odd(nxt):
            u = copy.deepcopy(steady)
            dpm = u.datapath_config[blk_mul2]
            dpm.op = AluOp.BYPASS  # x1 = Src0 (skip *C0)
            dps = u.datapath_config[blk_sub]
            dps.op = AluOp.BYPASS
            dps.alu_src1 = dps.alu_src0  # x2 = x1 (skip -1)
            u.trigger = (Trigger.SRC_TENSOR_DONE, Trigger.COUNT, Trigger.NONE)
            u.next_uop = (0, nxt, 0)
            u.repeat_count = 1
            return u

        prog = [mk_even(1), mk_odd(2), mk_even(1)]
        dos = DveOpSpec(name=name, opcode=row_idx, uops=prog, rd1_en=True)
        programs[ver] = dos
        shas[ver] = dos.sha(ver)

    final = _PatchedDveOp(name, spec, subdim=False, uops_sha=shas,
                          programs=programs)
    OPS[-1] = final
    from concourse.dve_table_gen import free_opcode_rows

    assert row_idx in free_opcode_rows("TRN2"), (name, row_idx)
    return final


LSTM_SCAN_OP = _register_lstm_scan_op() if SCAN else None

# torch gate block (i,f,g,o) -> our slot; GPRE = slot of the pre-doubled g
if SCAN:
    PERM, GPRE = [0, 1, 2, 3], 2
else:
    PERM, GPRE = [0, 1, 3, 2], 3


def _build(T):
    ngrp = T // SPB
    ch = min(CH, T)
    HD = BF16 if H_BF16 else F32
    XD = BF16 if X_BF16 else F32
    nc = bacc.Bacc("TRN2", target_bir_lowering=False, debug=False)

    xT = nc.declare_dram_parameter("xT", [H, KT, T, BC], XD, isOutput=False)
    whhT = nc.declare_dram_parameter("whhT", [H, G4, H], HD, isOutput=False)
    wihT = nc.declare_dram_parameter("wihT", [H, KT, G4, H], XD, isOutput=False)
    bias4 = nc.declare_dram_parameter("bias4", [G4, H], XD, isOutput=False)
    ind4 = nc.declare_dram_parameter("ind4", [G4, SPB * G4 * BC], XD, isOutput=False)
    wdT = nc.declare_dram_parameter("wdT", [H, OUT], F32, isOutput=False)
    bd = nc.declare_dram_parameter("bd", [1, OUT], F32, isOutput=False)
    out = nc.declare_dram_parameter("out", [BC, OUT], F32, isOutput=True)

    NSPLIT = 512  # dense tail: first PSUM bank columns
    NREST = OUT - NSPLIT

    with tile.TileContext(nc) as tc:
        with (
            tc.tile_pool(name="const", bufs=1) as constp,
            tc.tile_pool(name="xs", bufs=6) as xpool,
            tc.tile_pool(name="state", bufs=1) as state,
            tc.tile_pool(name="work", bufs=3) as work,
        ):
            whh_s = constp.tile([H, G4, H], HD)
            wih_s = constp.tile([H, KT, G4, H], XD)
            bias_s = constp.tile([G4, H], XD)
            ind_s = constp.tile([G4, SPB * G4 * BC], XD)
            wd_s = constp.tile([H, OUT], F32)
            bd_s = constp.tile([1, OUT], F32)
            ones1 = constp.tile([1, BC], F32)
            nc.gpsimd.dma_start(whh_s[:], whhT[:])
            nc.gpsimd.dma_start(wih_s[:], wihT[:])
            nc.gpsimd.dma_start(bias_s[:], bias4[:])
            nc.gpsimd.dma_start(ind_s[:], ind4[:])
            nc.gpsimd.dma_start(wd_s[:], wdT[:])
            nc.gpsimd.dma_start(bd_s[:], bd[:])
            nc.vector.memset(ones1[:], 1.0)

            # persistent state: h transposed [H, BC].
            # W = [sig(i) sig(f) sig(o) sig(2g) | c]: the sigmoid of all 4
            # (pre-scaled) gates lands in W[:,0:128] right next to the cell
            # state c in W[:,128:160], so [i|f] (x) [sig2g|c] is one
            # contiguous 64-wide multiply. tanh(g) = 2*sig(2g)-1 is folded
            # into the cell update (g weights are pre-doubled on the host).
            # (A 2-op cell update via a duplicated-sigmoid scatter measured
            # dead even on HW: the saved DVE op's fixed cost reappears in
            # the 2x-wider activation. This layout is the local optimum.)
            hT = state.tile([H, BC], HD)
            hT32 = state.tile([H, BC], F32)
            # SCAN mode: two ping-pong cell sets, each 4 blocks of 2*BC
            # (i, f, sg/c, o) with gate values at even offsets and the cell
            # state c at the odd offsets of the sg block. Otherwise the flat
            # [i f o sg | c] layout.
            W = state.tile([H, 16 * BC] if SCAN else [H, 5 * BC], F32)

            nchunk = (T + ch - 1) // ch
            xtiles = [None] * nchunk

            def ensure_chunk(ci):
                if xtiles[ci] is None:
                    xt = xpool.tile([H, KT, ch, BC], XD)
                    nc.gpsimd.dma_start(
                        xt[:], xT[:, :, ci * ch : (ci + 1) * ch, :]
                    )
                    xtiles[ci] = xt

            for _rep in range(REP):
              if True:
                xtiles = [None] * nchunk
                nc.vector.memset(hT[:], 0.0)
                nc.vector.memset(W[:], 0.0)
                psump_cm = tc.tile_pool(name=f"psum{_rep}", bufs=AHEAD + 2, space="PSUM")
                psump = psump_cm.__enter__()
                pstiles = [None] * ngrp

                from concourse.tile import add_dep_helper

                def alloc_group(g, dep=None):
                    # allocate the PSUM bank for group g and seed it with the
                    # bias: the ONE start=True matmul covering the whole bank
                    # (start=True clears has_written bank-wide, so it must be
                    # the single first writer; everything after accumulates)
                    t0 = g * SPB
                    ensure_chunk(t0 // ch)
                    ps = psump.tile([H, SPB, G4, BC], F32)
                    pstiles[g] = ps
                    mm = nc.tensor.matmul(
                        ps[:].rearrange("p a g b -> p (a g b)"),
                        bias_s[:],
                        ind_s[:],
                        start=True,
                        stop=False,
                        skip_group_check=True,
                    )
                    if dep is not None:
                        add_dep_helper(mm.ins, dep.ins, reason="xproj window anchor")

                def emit_xproj_mms(g, pairs, dep=None):
                    # accumulate W_ih*x contributions (gi, kt) for group g
                    t0 = g * SPB
                    ci = t0 // ch
                    xt = xtiles[ci]
                    s0 = t0 - ci * ch
                    ps = pstiles[g]
                    for gi, kt in pairs:
                        mm = nc.tensor.matmul(
                            ps[:, :, gi, :],
                            wih_s[:, kt, gi, :],
                            xt[:, kt, s0 : s0 + SPB, :],
                            start=False,
                            stop=False,
                            skip_group_check=True,
                        )
                        if dep is not None:
                            add_dep_helper(mm.ins, dep.ins, reason="xproj window anchor")

                ALL_PAIRS = [(gi, kt) for gi in range(G4) for kt in range(KT)]

                for g in range(min(AHEAD, ngrp)):
                    alloc_group(g)
                    emit_xproj_mms(g, ALL_PAIRS)

                # next group's xproj is spread across this group's four
                # steps, each sub-batch dep-anchored on that step's sigmoid
                # so the scheduler places it in the step's PE-idle window
                # (one 9-op batch at a group boundary overflows the window)
                XPIECES = [[], ALL_PAIRS[0:3], ALL_PAIRS[3:6], ALL_PAIRS[6:8]]
                for g in range(ngrp):
                    ps = pstiles[g]
                    for s in range(SPB):
                        t_glob = g * SPB + s
                        # W_hh * h into the gate bank (critical path).
                        for gi in ((0, 1, 2, 3) if SCAN else (3, 0, 1, 2)):
                            nc.tensor.matmul(
                                ps[:, s, gi, :],
                                whh_s[:, gi, :],
                                hT[:],
                                start=False,
                                stop=(gi == (3 if SCAN else 2)),
                                skip_group_check=True,
                            )
                        tct = work.tile([H, BC], F32)
                        ht_dst = hT32 if t_glob == T - 1 else hT
                        if SCAN:
                            Xb = 0 if (t_glob % 2 == 0) else 8 * BC
                            Yb = 8 * BC - Xb
                            if SIG3:
                                # sig(i,f,g) only waits on the first 3 W_hh
                                # matmuls; sig(o) runs behind it on ScalarE,
                                # hidden under the DVE cell op (o is not
                                # needed until the h-mul)
                                sig_inst = nc.scalar.activation(
                                    W[:, Xb : Xb + 6 * BC].rearrange(
                                        "p (g b two) -> p g b two", g=3, two=2
                                    )[:, :, :, 0],
                                    ps[:, s, 0:3, :],
                                    mybir.ActivationFunctionType.Sigmoid,
                                )
                                nc.scalar.activation(
                                    W[:, Xb + 6 * BC : Xb + 8 * BC].rearrange(
                                        "p (b two) -> p b two", two=2
                                    )[:, :, 0],
                                    ps[:, s, 3, :],
                                    mybir.ActivationFunctionType.Sigmoid,
                                )
                            else:
                                # sigmoid of all 4 gates -> even offsets of X
                                sig_inst = nc.scalar.activation(
                                    W[:, Xb : Xb + 8 * BC].rearrange(
                                        "p (g b two) -> p g b two", g=4, two=2
                                    )[:, :, :, 0],
                                    ps[:, s, :, :],
                                    mybir.ActivationFunctionType.Sigmoid,
                                )
                            # whole cell update in one segmented-scan DVE op:
                            # set Y's sg/c block gets [junk | c_new] pairs
                            in1 = W[:, Xb : Xb + 4 * BC].rearrange(
                                "p (j b two) -> p j b two", j=2, two=2
                            )[:, :, :, 0].rearrange("p j b -> p b j")
                            nc.vector._custom_dve(
                                LSTM_SCAN_OP,
                                out=W[:, Yb + 4 * BC : Yb + 6 * BC],
                                in0=W[:, Xb + 4 * BC : Xb + 6 * BC],
                                in1=in1,
                                s0=2.0,
                            )
                            nc.scalar.activation(
                                tct[:],
                                W[:, Yb + 4 * BC : Yb + 6 * BC].rearrange(
                                    "p (b two) -> p b two", two=2
                                )[:, :, 1],
                                mybir.ActivationFunctionType.Tanh,
                            )
                            nc.vector.tensor_mul(
                                ht_dst[:],
                                W[:, Xb + 6 * BC : Xb + 8 * BC].rearrange(
                                    "p (b two) -> p b two", two=2
                                )[:, :, 0],
                                tct[:],
                            )
                        else:
                            prod = work.tile([H, 2 * BC], F32)
                            # sigmoid of all 4 gates (g pre-scaled by 2)
                            sig_inst = nc.scalar.activation(
                                W[:, 0 : 4 * BC].rearrange("p (g b) -> p g b", g=4),
                                ps[:, s, :, :],
                                mybir.ActivationFunctionType.Sigmoid,
                            )
                            if FUSED:
                                # fused DVE op: prod = [i*(2*sig2g-1) | f*c]
                                nc.vector._custom_dve(
                                    LSTM_CELL_OP,
                                    out=prod[:].rearrange("p (s b) -> p s b", s=2),
                                    in0=W[:, 3 * BC : 5 * BC].rearrange(
                                        "p (s b) -> p s b", s=2
                                    ),
                                    in1=W[:, 0 : 2 * BC].rearrange(
                                        "p (s b) -> p s b", s=2
                                    ),
                                )
                                # c = i*tanh(g) + f*c
                                nc.vector.tensor_add(
                                    W[:, 4 * BC : 5 * BC],
                                    prod[:, 0:BC],
                                    prod[:, BC : 2 * BC],
                                )
                            else:
                                # prod = [i*sig2g | f*c]
                                nc.vector.tensor_mul(
                                    prod[:], W[:, 0 : 2 * BC], W[:, 3 * BC : 5 * BC]
                                )
                                # c = 2*prod0 - i + prod1
                                nc.vector.scalar_tensor_tensor(
                                    tct[:], prod[:, 0:BC], 2.0, W[:, 0:BC],
                                    op0=mybir.AluOpType.mult,
                                    op1=mybir.AluOpType.subtract,
                                )
                                nc.vector.tensor_add(
                                    W[:, 4 * BC : 5 * BC], tct[:],
                                    prod[:, BC : 2 * BC],
                                )
                            nc.scalar.activation(
                                tct[:],
                                W[:, 4 * BC : 5 * BC],
                                mybir.ActivationFunctionType.Tanh,
                            )
                            nc.vector.tensor_mul(
                                ht_dst[:], W[:, 2 * BC : 3 * BC], tct[:]
                            )
                        if g + AHEAD < ngrp:
                            if s == 0:
                                alloc_group(g + AHEAD, dep=sig_inst)
                            emit_xproj_mms(g + AHEAD, XPIECES[s], dep=sig_inst)
                    pstiles[g] = None

                psump_cm.__exit__(None, None, None)
            # dense + softmax tail
            with tc.tile_pool(name="psd", bufs=2, space="PSUM") as psumd:
                lA = psumd.tile([BC, NSPLIT], F32)
                lB = psumd.tile([BC, NREST], F32)
                # bias matmuls first: they don't depend on the last hidden
                # state, so they run during the final step's elementwise ops
                nc.tensor.matmul(
                    lA[:], ones1[:], bd_s[:, 0:NSPLIT], start=True, stop=False,
                    skip_group_check=True,
                )
                nc.tensor.matmul(
                    lB[:], ones1[:], bd_s[:, NSPLIT:OUT], start=True, stop=False,
                    skip_group_check=True,
                )
                nc.tensor.matmul(
                    lA[:], hT32[:], wd_s[:, 0:NSPLIT], start=False, stop=True,
                    skip_group_check=True,
                )
                nc.tensor.matmul(
                    lB[:], hT32[:], wd_s[:, NSPLIT:OUT], start=False, stop=True,
                    skip_group_check=True,
                )
                mA = work.tile([BC, 1], F32)
                mB = work.tile([BC, 1], F32)
                mneg = work.tile([BC, 1], F32)
                sA = work.tile([BC, 1], F32)
                sB = work.tile([BC, 1], F32)
                stot = work.tile([BC, 1], F32)
                rec = work.tile([BC, 1], F32)
                sm = work.tile([BC, OUT], F32)
                nc.vector.reduce_max(mA[:], lA[:], axis=mybir.AxisListType.X)
                nc.vector.reduce_max(mB[:], lB[:], axis=mybir.AxisListType.X)
                nc.vector.tensor_max(mA[:], mA[:], mB[:])
                nc.vector.tensor_scalar_mul(mneg[:], mA[:], -1.0)
                nc.scalar.activation(
                    sm[:, 0:NSPLIT], lA[:], mybir.ActivationFunctionType.Exp,
                    bias=mneg[:], accum_out=sA[:],
                )
                nc.scalar.activation(
                    sm[:, NSPLIT:OUT], lB[:], mybir.ActivationFunctionType.Exp,
                    bias=mneg[:], accum_out=sB[:],
                )
                nc.vector.tensor_add(stot[:], sA[:], sB[:])
                nc.vector.reciprocal(rec[:], stot[:])
                nc.vector.tensor_scalar_mul(sm[:], sm[:], rec[:])
                nc.gpsimd.dma_start(out[:], sm[:])

    nc.compile()
    return nc


def _get_nc(T):
    key = (T, REP, H_BF16, X_BF16, FUSED, SCAN, SIG3)
    if key not in _CACHE:
        _CACHE[key] = _build(T)
    return _CACHE[key]


def prep_inputs(x, w_ih, w_hh, b_ih, b_hh, w_dense, b_dense):
    import ml_dtypes
    xd = ml_dtypes.bfloat16 if X_BF16 else np.float32
    hd = ml_dtypes.bfloat16 if H_BF16 else np.float32
    T = x.shape[0]
    x = np.ascontiguousarray(x, dtype=np.float32)
    # xT[k, kt, t, b] = x[t, b, kt*128+k]
    xt_all = np.ascontiguousarray(
        x.reshape(T, B, KT, H).transpose(3, 2, 0, 1).astype(xd)
    )
    whhT = np.ascontiguousarray(
        w_hh.reshape(4, H, H)[PERM].transpose(2, 0, 1).astype(hd)
    )
    wihT = np.ascontiguousarray(
        w_ih.reshape(4, H, KT, H)[PERM].transpose(3, 2, 0, 1).astype(xd)
    )
    bias4 = (b_ih + b_hh).reshape(4, H)[PERM].astype(np.float32)
    # pre-scale the g gate by 2: tanh(x) = 2*sigmoid(2x) - 1
    whhT = whhT.copy(); wihT = wihT.copy()
    whhT[:, GPRE, :] = whhT[:, GPRE, :] * np.asarray(2.0, whhT.dtype)
    wihT[:, :, GPRE, :] = wihT[:, :, GPRE, :] * np.asarray(2.0, wihT.dtype)
    bias4[GPRE] *= 2.0
    bias4 = np.ascontiguousarray(bias4.astype(xd))
    # ind4[g, n] for n = s*(G4*BC) + gq*BC + b  -> 1.0 iff gq == g
    ind4 = np.zeros((G4, SPB * G4 * BC), dtype=xd)
    nidx = np.arange(SPB * G4 * BC)
    gq = (nidx // BC) % G4
    for g in range(G4):
        ind4[g, gq == g] = 1.0
    wdT = np.ascontiguousarray(w_dense.T, dtype=np.float32)
    bd = np.ascontiguousarray(b_dense.reshape(1, OUT), dtype=np.float32)

    in_maps = []
    for c in range(N_CORES):
        in_maps.append(
            {
                "xT": np.ascontiguousarray(xt_all[:, :, :, c * BC : (c + 1) * BC]),
                "whhT": whhT,
                "wihT": wihT,
                "bias4": bias4,
                "ind4": ind4,
                "wdT": wdT,
                "bd": bd,
            }
        )
    return in_maps


def kernel(x, w_ih, w_hh, b_ih, b_hh, w_dense, b_dense):
    x = np.asarray(x)
    T = x.shape[0]
    nc = _get_nc(T)
    in_maps = prep_inputs(
        np.asarray(x), np.asarray(w_ih), np.asarray(w_hh),
        np.asarray(b_ih), np.asarray(b_hh),
        np.asarray(w_dense), np.asarray(b_dense),
    )
    res = run_bass_kernel_spmd(nc, in_maps, list(range(N_CORES)))
    return np.concatenate(
        [res.results[c]["out"] for c in range(N_CORES)], axis=0
    ).astype(np.float32)



# revision 35
# speedup vs baseline: 1.2056x; 1.0092x over previous
"""Trainium2 Bass kernel for the LSTM+dense+softmax model.

Model (see reference): x[T=512, B=256, IN=256] -> LSTM(H=128) last hidden
-> dense(OUT=1000) -> softmax. Data-parallel over batch across 8 cores
(32 batch elements per core), weights replicated.

Layout: recurrent state is kept transposed [H=128 partitions, batch] so the
per-step W_hh matmuls, gate nonlinearities and cell update all run at full
partition width with no transposes. Gate pre-activations for 4 consecutive
steps share one PSUM bank: W_ih*x contributions (+bias) are accumulated
ahead of time, W_hh*h is added when the step arrives, and ScalarE applies
sigmoid directly out of PSUM. Both matmul paths run in bfloat16.

The recurrence is LATENCY-bound: 512 serial steps, each a cross-engine
dependency chain whose cost is per-instruction fixed overheads (ScalarE op
~300ns, DVE op ~170ns, PE SBUF fill ~180ns, ~35ns/semaphore hop), not
data width. Optimizations here, measured per step on hardware:
  - one 4-gate sigmoid (g pre-doubled; tanh(g)=2*sig(2g)-1 folded into the
    cell update) -- a split (i,f,g)+(o) sigmoid measured slower twice.
  - the ENTIRE cell update c=i*tanh(g)+f*c is ONE custom DVE instruction
    (LSTM_SCAN_ANT): a hand-built 3-uop ping-pong FSM over interleaved
    (even,odd) element pairs -- even elements compute i*(2sg-1) resetting
    the running sum, odd elements add f*c and emit c_new. Gate/cell values
    live interleaved in two ping-pong "sets" so every operand of every op
    stays a single affine access pattern. (The stock DSL has no segmented
    scan; the uop program is derived by patching a lowered scan template.)
  - x-projection matmuls + the bias matmul are dep-anchored (add_dep_helper
    on each step's sigmoid) so the simulation-guided scheduler spreads them
    into per-step PE-idle windows; emission-order/priority placement alone
    measured +280..560ns spikes on group-boundary steps.
  - dense-layer bias matmuls precede the h matmuls (no dependency on h),
    small x chunks (CH=8) + AHEAD=2 priming cut kernel head time.
Step ~1650ns: sig4(374) SCAN(240) tanh(321) hmul(191) 4x W_hh mm(268) +
sem hops. Measured slower/dead: sigmoid split, per-element DVE ops (3-op
cell), Pool offload, emission-order xproj placement, fp32 matmul paths.
"""

import numpy as np

import concourse.bacc as bacc
import concourse.mybir as mybir
import concourse.tile as tile
from concourse.bass_utils import run_bass_kernel_spmd

SEQ = 512
B = 256
IN = 256
H = 128
OUT = 1000
N_CORES = 8
BC = B // N_CORES  # 32 batch per core
KT = IN // H  # 2 k-tiles for the input projection
G4 = 4  # gate slot order: (i,f,g,o) in SCAN mode, (i,f,o,g) otherwise
SPB = 4  # steps per PSUM bank group (4*4*32 fp32 = one 2KB bank)
AHEAD = 2  # bank groups of x-projection lookahead
CH = 8  # timesteps per streamed x chunk (small first chunks cut startup)

F32 = mybir.dt.float32
BF16 = mybir.dt.bfloat16

import os as _os
H_BF16 = _os.environ.get("LSTM_H_BF16", "1") == "1"  # W_hh*h path in bf16
X_BF16 = _os.environ.get("LSTM_X_BF16", "1") == "1"  # W_ih*x (+bias) path in bf16
REP = int(_os.environ.get("LSTM_REP", "1"))  # timing amplification (bench only)
FUSED = _os.environ.get("LSTM_FUSED", "1") == "1"  # custom-DVE fused cell update

_CACHE = {}


# --- custom DVE op: fused LSTM cell products ------------------------------
# One instruction computing, over [P, 2, N] paged operands,
#   page 0: out = i * (2*sg - 1)   (= i * tanh(g~), sg = sigmoid(2*g~))
#   page 1: out = f * c
# replacing the stock MULTIPLY + MULTIPLY,SUBTRACT pair on the recurrence's
# serial critical path. Registered into concourse.dve_ops.OPS at import time
# (the documented per-NEFF extension point; the uop table is regenerated for
# every compile, no firmware change involved).
def _lstm_cell_ref(in0, in1, s0, s1, imm2):
    a = in0.reshape((in0.shape[0], 2, -1)).astype(np.float32)
    b = in1.reshape(a.shape).astype(np.float32)
    out = np.empty_like(a)
    out[:, 0, :] = b[:, 0, :] * (2.0 * a[:, 0, :] - 1.0)
    out[:, 1, :] = b[:, 1, :] * a[:, 1, :]
    return out.reshape(in0.shape)


def _register_lstm_cell_op():
    import re
    import concourse.dve_ops as dve_ops_mod
    from concourse.dve_ops import OPS, CUSTOM_DVE_SPECS, DveOp
    from concourse.dve_spec import Spec, Src0, Src1, One, SubIdx

    name = "LSTM_CELL_ANT"
    for op in OPS:
        if op.name == name:
            return op
    pg0 = SubIdx < One  # 1.0 on page 0, 0.0 on page 1
    spec = Spec(
        body=Src1 * (Src0 * (One + pg0) - pg0),
        reference=_lstm_cell_ref,
    )
    probe = DveOp(name, spec, subdim=True, uops_sha={})
    OPS.append(probe)
    dve_ops_mod._SUB_OPCODE_FOR_NAME[name] = (
        dve_ops_mod._CUSTOM_DVE_ROW_BASE + len(OPS) - 1
    )
    CUSTOM_DVE_SPECS[name] = spec
    shas = {}
    for ver in ("v3", "v4"):
        try:
            probe.compile(ver)
            shas[ver] = probe.uops_sha.get(ver, "")
        except ValueError as e:
            m = re.search(r'"([0-9a-f]{16})"', str(e))
            if not m:
                raise
            shas[ver] = m.group(1)
    final = DveOp(name, spec, subdim=True, uops_sha=shas)
    OPS[-1] = final
    from concourse.dve_table_gen import free_opcode_rows

    row = dve_ops_mod.get_dve_sub_opcode(name)
    assert row in free_opcode_rows("TRN2"), (name, row)
    return final


LSTM_CELL_OP = _register_lstm_cell_op() if FUSED else None

SCAN = _os.environ.get("LSTM_SCAN", "1") == "1"  # segmented-scan cell update
# split sigmoid (i,f,g)+(o): measured ~70ns/step SLOWER than one 4-gate
# sigmoid even with dep-anchored xproj — the second Act op's occupancy
# outweighs the earlier start. Keep off.
SIG3 = _os.environ.get("LSTM_SIG3", "0") == "1"


# --- custom DVE op: full cell update in ONE instruction -------------------
# Streams interleaved pairs (even, odd) = ((sg_b, c_b) via in0, (i_b, f_b)
# via in1) and emits at each odd position  c_new_b = i_b*(2*sg_b-1) + f_b*c_b.
# The stock DSL has no per-pair scan reset, so the uop program is hand-built:
# a 3-state ping-pong FSM (even resets the running sum to i*(2sg-1), odd adds
# f*c), derived by patching the lowered template of a plain scan spec.
def _lstm_scan_ref(in0, in1, s0, s1, imm2):
    P = in0.shape[0]
    a = in0.reshape(P, -1, 2).astype(np.float32)  # sg, c
    b = np.asarray(in1).reshape(P, -1, 2).astype(np.float32)  # i, f
    c0 = s0 if isinstance(s0, float) else s0.reshape(P, 1)
    it = b[:, :, 0] * (c0 * a[:, :, 0] - 1.0)
    fc = b[:, :, 1] * a[:, :, 1]
    out = np.empty_like(a)
    out[:, :, 0] = it
    out[:, :, 1] = it + fc
    return out.reshape(in0.shape)


def _register_lstm_scan_op():
    import copy
    import concourse.dve_ops as dve_ops_mod
    from concourse.dve_ops import OPS, CUSTOM_DVE_SPECS, DveOp
    from concourse.dve_spec import Spec, Src0, Src1, One, C0, scan, lower, AluOp
    from concourse.dve_uop import DveOpSpec, Trigger
    from dataclasses import dataclass

    name = "LSTM_SCAN_ANT"
    for op in OPS:
        if op.name == name:
            return op

    # template: x = (Src0*C0 - 1)*Src1 ; S = scan(ADD, x)
    x = ((Src0 * C0) - One) * Src1
    spec = Spec(body=scan(AluOp.ADD, x), reference=_lstm_scan_ref)

    @dataclass(frozen=True)
    class _PatchedDveOp(DveOp):
        programs: dict = None

        def compile(self, ver):
            return self.programs[ver]

    programs = {}
    shas = {}
    OPS.append(None)  # reserve the row before computing opcode
    row_idx = dve_ops_mod._CUSTOM_DVE_ROW_BASE + len(OPS) - 1
    dve_ops_mod._SUB_OPCODE_FOR_NAME[name] = row_idx
    CUSTOM_DVE_SPECS[name] = spec
    for ver in ("v3", "v4"):
        uops = lower(spec, ver=ver)
        assert len(uops) == 2, len(uops)
        steady = uops[1]
        # identify blocks: blk_mul2 (MUL Src0*C0), blk_sub (-1), blk_scan (ADD CURR)
        blk_mul2 = blk_sub = blk_scan = None
        for i, dp in enumerate(steady.datapath_config):
            if dp.op == AluOp.MULTIPLY and blk_mul2 is None:
                blk_mul2 = i
            elif dp.op == AluOp.SUBTRACT and blk_sub is None:
                blk_sub = i
            elif dp.op == AluOp.ADD and "CURR" in dp.alu_src0.name:
                blk_scan = i
        assert None not in (blk_mul2, blk_sub, blk_scan), (
            blk_mul2, blk_sub, blk_scan)

        def mk_even(nxt):
            u = copy.deepcopy(steady)
            dp = u.datapath_config[blk_scan]
            dp.op = AluOp.BYPASS
            dp.alu_src0 = dp.alu_src1  # pass x through; resets running sum
            u.trigger = (Trigger.SRC_TENSOR_DONE, Trigger.COUNT, Trigger.NONE)
            u.next_uop = (0, nxt, 0)
            u.repeat_count = 1
            return u

        def mk_odd(nxt):
            u = copy.deepcopy(steady)
            dpm = u.datapath_config[blk_mul2]
            dpm.op = AluOp.BYPASS  # x1 = Src0 (skip *C0)
            dps = u.datapath_config[blk_sub]
            dps.op = AluOp.BYPASS
            dps.alu_src1 = dps.alu_src0  # x2 = x1 (skip -1)
            u.trigger = (Trigger.SRC_TENSOR_DONE, Trigger.COUNT, Trigger.NONE)
            u.next_uop = (0, nxt, 0)
            u.repeat_count = 1
            return u

        prog = [mk_even(1), mk_odd(2), mk_even(1)]
        dos = DveOpSpec(name=name, opcode=row_idx, uops=prog, rd1_en=True)
        programs[ver] = dos
        shas[ver] = dos.sha(ver)

    final = _PatchedDveOp(name, spec, subdim=False, uops_sha=shas,
                          programs=programs)
    OPS[-1] = final
    from concourse.dve_table_gen import free_opcode_rows

    assert row_idx in free_opcode_rows("TRN2"), (name, row_idx)
    return final


LSTM_SCAN_OP = _register_lstm_scan_op() if SCAN else None

# torch gate block (i,f,g,o) -> our slot; GPRE = slot of the pre-doubled g
if SCAN:
    PERM, GPRE = [0, 1, 2, 3], 2
else:
    PERM, GPRE = [0, 1, 3, 2], 3


def _build(T):
    ngrp = T // SPB
    ch = min(CH, T)
    HD = BF16 if H_BF16 else F32
    XD = BF16 if X_BF16 else F32
    nc = bacc.Bacc("TRN2", target_bir_lowering=False, debug=False)

    xT = nc.declare_dram_parameter("xT", [H, KT, T, BC], XD, isOutput=False)
    whhT = nc.declare_dram_parameter("whhT", [H, G4, H], HD, isOutput=False)
    wihT = nc.declare_dram_parameter("wihT", [H, KT, G4, H], XD, isOutput=False)
    bias4 = nc.declare_dram_parameter("bias4", [G4, H], XD, isOutput=False)
    ind4 = nc.declare_dram_parameter("ind4", [G4, SPB * G4 * BC], XD, isOutput=False)
    wdT = nc.declare_dram_parameter("wdT", [H, OUT], BF16, isOutput=False)
    bd = nc.declare_dram_parameter("bd", [1, OUT], BF16, isOutput=False)
    out = nc.declare_dram_parameter("out", [BC, OUT], F32, isOutput=True)

    NSPLIT = 512  # dense tail: first PSUM bank columns
    NREST = OUT - NSPLIT

    with tile.TileContext(nc) as tc:
        with (
            tc.tile_pool(name="const", bufs=1) as constp,
            tc.tile_pool(name="xs", bufs=6) as xpool,
            tc.tile_pool(name="state", bufs=1) as state,
            tc.tile_pool(name="work", bufs=3) as work,
        ):
            whh_s = constp.tile([H, G4, H], HD)
            wih_s = constp.tile([H, KT, G4, H], XD)
            bias_s = constp.tile([G4, H], XD)
            ind_s = constp.tile([G4, SPB * G4 * BC], XD)
            wd_s = constp.tile([H, OUT], BF16)
            bd_s = constp.tile([1, OUT], BF16)
            ones1 = constp.tile([1, BC], BF16)
            nc.gpsimd.dma_start(whh_s[:], whhT[:])
            nc.gpsimd.dma_start(wih_s[:], wihT[:])
            nc.gpsimd.dma_start(bias_s[:], bias4[:])
            nc.gpsimd.dma_start(ind_s[:], ind4[:])
            nc.vector.memset(ones1[:], 1.0)

            # persistent state: h transposed [H, BC].
            # W = [sig(i) sig(f) sig(o) sig(2g) | c]: the sigmoid of all 4
            # (pre-scaled) gates lands in W[:,0:128] right next to the cell
            # state c in W[:,128:160], so [i|f] (x) [sig2g|c] is one
            # contiguous 64-wide multiply. tanh(g) = 2*sig(2g)-1 is folded
            # into the cell update (g weights are pre-doubled on the host).
            # (A 2-op cell update via a duplicated-sigmoid scatter measured
            # dead even on HW: the saved DVE op's fixed cost reappears in
            # the 2x-wider activation. This layout is the local optimum.)
            hT = state.tile([H, BC], HD)
            hT32 = state.tile([H, BC], BF16)
            # SCAN mode: two ping-pong cell sets, each 4 blocks of 2*BC
            # (i, f, sg/c, o) with gate values at even offsets and the cell
            # state c at the odd offsets of the sg block. Otherwise the flat
            # [i f o sg | c] layout.
            W = state.tile([H, 16 * BC] if SCAN else [H, 5 * BC], F32)

            nchunk = (T + ch - 1) // ch
            xtiles = [None] * nchunk

            def ensure_chunk(ci):
                if xtiles[ci] is None:
                    xt = xpool.tile([H, KT, ch, BC], XD)
                    nc.gpsimd.dma_start(
                        xt[:], xT[:, :, ci * ch : (ci + 1) * ch, :]
                    )
                    xtiles[ci] = xt

            for _rep in range(REP):
              if True:
                xtiles = [None] * nchunk
                nc.vector.memset(hT[:], 0.0)
                nc.vector.memset(W[:], 0.0)
                psump_cm = tc.tile_pool(name=f"psum{_rep}", bufs=AHEAD + 2, space="PSUM")
                psump = psump_cm.__enter__()
                pstiles = [None] * ngrp

                from concourse.tile import add_dep_helper

                def alloc_group(g, dep=None):
                    # allocate the PSUM bank for group g and seed it with the
                    # bias: the ONE start=True matmul covering the whole bank
                    # (start=True clears has_written bank-wide, so it must be
                    # the single first writer; everything after accumulates)
                    t0 = g * SPB
                    ensure_chunk(t0 // ch)
                    ps = psump.tile([H, SPB, G4, BC], F32)
                    pstiles[g] = ps
                    mm = nc.tensor.matmul(
                        ps[:].rearrange("p a g b -> p (a g b)"),
                        bias_s[:],
                        ind_s[:],
                        start=True,
                        stop=False,
                        skip_group_check=True,
                    )
                    if dep is not None:
                        add_dep_helper(mm.ins, dep.ins, reason="xproj window anchor")

                def emit_xproj_mms(g, pairs, dep=None):
                    # accumulate W_ih*x contributions (gi, kt) for group g
                    t0 = g * SPB
                    ci = t0 // ch
                    xt = xtiles[ci]
                    s0 = t0 - ci * ch
                    ps = pstiles[g]
                    for gi, kt in pairs:
                        mm = nc.tensor.matmul(
                            ps[:, :, gi, :],
                            wih_s[:, kt, gi, :],
                            xt[:, kt, s0 : s0 + SPB, :],
                            start=False,
                            stop=False,
                            skip_group_check=True,
                        )
                        if dep is not None:
                            add_dep_helper(mm.ins, dep.ins, reason="xproj window anchor")

                ALL_PAIRS = [(gi, kt) for gi in range(G4) for kt in range(KT)]

                for g in range(min(AHEAD, ngrp)):
                    alloc_group(g)
                    emit_xproj_mms(g, ALL_PAIRS)

                # next group's xproj is spread across this group's four
                # steps, each sub-batch dep-anchored on that step's sigmoid
                # so the scheduler places it in the step's PE-idle window
                # (one 9-op batch at a group boundary overflows the window)
                XPIECES = [[], ALL_PAIRS[0:3], ALL_PAIRS[3:6], ALL_PAIRS[6:8]]
                for g in range(ngrp):
                    ps = pstiles[g]
                    for s in range(SPB):
                        t_glob = g * SPB + s
                        # W_hh * h into the gate bank (critical path).
                        for gi in ((0, 1, 2, 3) if SCAN else (3, 0, 1, 2)):
                            nc.tensor.matmul(
                                ps[:, s, gi, :],
                                whh_s[:, gi, :],
                                hT[:],
                                start=False,
                                stop=(gi == (3 if SCAN else 2)),
                                skip_group_check=True,
                            )
                        tct = work.tile([H, BC], F32)
                        ht_dst = hT32 if t_glob == T - 1 else hT
                        if SCAN:
                            Xb = 0 if (t_glob % 2 == 0) else 8 * BC
                            Yb = 8 * BC - Xb
                            if SIG3:
                                # sig(i,f,g) only waits on the first 3 W_hh
                                # matmuls; sig(o) runs behind it on ScalarE,
                                # hidden under the DVE cell op (o is not
                                # needed until the h-mul)
                                sig_inst = nc.scalar.activation(
                                    W[:, Xb : Xb + 6 * BC].rearrange(
                                        "p (g b two) -> p g b two", g=3, two=2
                                    )[:, :, :, 0],
                                    ps[:, s, 0:3, :],
                                    mybir.ActivationFunctionType.Sigmoid,
                                )
                                nc.scalar.activation(
                                    W[:, Xb + 6 * BC : Xb + 8 * BC].rearrange(
                                        "p (b two) -> p b two", two=2
                                    )[:, :, 0],
                                    ps[:, s, 3, :],
                                    mybir.ActivationFunctionType.Sigmoid,
                                )
                            else:
                                # sigmoid of all 4 gates -> even offsets of X
                                sig_inst = nc.scalar.activation(
                                    W[:, Xb : Xb + 8 * BC].rearrange(
                                        "p (g b two) -> p g b two", g=4, two=2
                                    )[:, :, :, 0],
                                    ps[:, s, :, :],
                                    mybir.ActivationFunctionType.Sigmoid,
                                )
                            # whole cell update in one segmented-scan DVE op:
                            # set Y's sg/c block gets [junk | c_new] pairs
                            in1 = W[:, Xb : Xb + 4 * BC].rearrange(
                                "p (j b two) -> p j b two", j=2, two=2
                            )[:, :, :, 0].rearrange("p j b -> p b j")
                            nc.vector._custom_dve(
                                LSTM_SCAN_OP,
                                out=W[:, Yb + 4 * BC : Yb + 6 * BC],
                                in0=W[:, Xb + 4 * BC : Xb + 6 * BC],
                                in1=in1,
                                s0=2.0,
                            )
                            tanh_inst = nc.scalar.activation(
                                tct[:],
                                W[:, Yb + 4 * BC : Yb + 6 * BC].rearrange(
                                    "p (b two) -> p b two", two=2
                                )[:, :, 1],
                                mybir.ActivationFunctionType.Tanh,
                            )
                            nc.vector.tensor_mul(
                                ht_dst[:],
                                W[:, Xb + 6 * BC : Xb + 8 * BC].rearrange(
                                    "p (b two) -> p b two", two=2
                                )[:, :, 0],
                                tct[:],
                            )
                        else:
                            prod = work.tile([H, 2 * BC], F32)
                            # sigmoid of all 4 gates (g pre-scaled by 2)
                            sig_inst = nc.scalar.activation(
                                W[:, 0 : 4 * BC].rearrange("p (g b) -> p g b", g=4),
                                ps[:, s, :, :],
                                mybir.ActivationFunctionType.Sigmoid,
                            )
                            if FUSED:
                                # fused DVE op: prod = [i*(2*sig2g-1) | f*c]
                                nc.vector._custom_dve(
                                    LSTM_CELL_OP,
                                    out=prod[:].rearrange("p (s b) -> p s b", s=2),
                                    in0=W[:, 3 * BC : 5 * BC].rearrange(
                                        "p (s b) -> p s b", s=2
                                    ),
                                    in1=W[:, 0 : 2 * BC].rearrange(
                                        "p (s b) -> p s b", s=2
                                    ),
                                )
                                # c = i*tanh(g) + f*c
                                nc.vector.tensor_add(
                                    W[:, 4 * BC : 5 * BC],
                                    prod[:, 0:BC],
                                    prod[:, BC : 2 * BC],
                                )
                            else:
                                # prod = [i*sig2g | f*c]
                                nc.vector.tensor_mul(
                                    prod[:], W[:, 0 : 2 * BC], W[:, 3 * BC : 5 * BC]
                                )
                                # c = 2*prod0 - i + prod1
                                nc.vector.scalar_tensor_tensor(
                                    tct[:], prod[:, 0:BC], 2.0, W[:, 0:BC],
                                    op0=mybir.AluOpType.mult,
                                    op1=mybir.AluOpType.subtract,
                                )
                                nc.vector.tensor_add(
                                    W[:, 4 * BC : 5 * BC], tct[:],
                                    prod[:, BC : 2 * BC],
                                )
                            tanh_inst = nc.scalar.activation(
                                tct[:],
                                W[:, 4 * BC : 5 * BC],
                                mybir.ActivationFunctionType.Tanh,
                            )
                            nc.vector.tensor_mul(
                                ht_dst[:], W[:, 2 * BC : 3 * BC], tct[:]
                            )
                        if g + AHEAD < ngrp:
                            if s == 0:
                                alloc_group(g + AHEAD, dep=sig_inst)
                            emit_xproj_mms(g + AHEAD, XPIECES[s], dep=sig_inst)
                    pstiles[g] = None

                psump_cm.__exit__(None, None, None)
            # dense + softmax tail (weight DMAs were deferred so the
            # recurrence's x chunks got the DGE queue first)
            nc.gpsimd.dma_start(wd_s[:], wdT[:])
            nc.gpsimd.dma_start(bd_s[:], bd[:])
            with tc.tile_pool(name="psd", bufs=2, space="PSUM") as psumd:
                # preload the exp activation table during the dense matmuls
                # (otherwise the first EXP pays the 1.3us table load inline)
                warm_exp = work.tile([1, BC], F32)
                wx = nc.scalar.activation(
                    warm_exp[:], ones1[:], mybir.ActivationFunctionType.Exp
                )
                add_dep_helper(wx.ins, tanh_inst.ins, reason="exp table preload")
                lA = psumd.tile([BC, NSPLIT], F32)
                lB = psumd.tile([BC, NREST], F32)
                # bias matmuls first: they don't depend on the last hidden
                # state, so they run during the final step's elementwise ops
                nc.tensor.matmul(
                    lA[:], ones1[:], bd_s[:, 0:NSPLIT], start=True, stop=False,
                    skip_group_check=True,
                )
                nc.tensor.matmul(
                    lB[:], ones1[:], bd_s[:, NSPLIT:OUT], start=True, stop=False,
                    skip_group_check=True,
                )
                nc.tensor.matmul(
                    lA[:], hT32[:], wd_s[:, 0:NSPLIT], start=False, stop=True,
                    skip_group_check=True,
                )
                nc.tensor.matmul(
                    lB[:], hT32[:], wd_s[:, NSPLIT:OUT], start=False, stop=True,
                    skip_group_check=True,
                )
                mA = work.tile([BC, 1], F32)
                mB = work.tile([BC, 1], F32)
                mneg = work.tile([BC, 1], F32)
                sA = work.tile([BC, 1], F32)
                sB = work.tile([BC, 1], F32)
                stot = work.tile([BC, 1], F32)
                rec = work.tile([BC, 1], F32)
                sm = work.tile([BC, OUT], F32)
                nc.vector.reduce_max(mA[:], lA[:], axis=mybir.AxisListType.X)
                nc.vector.reduce_max(mB[:], lB[:], axis=mybir.AxisListType.X)
                nc.vector.tensor_max(mA[:], mA[:], mB[:])
                nc.vector.tensor_scalar_mul(mneg[:], mA[:], -1.0)
                nc.scalar.activation(
                    sm[:, 0:NSPLIT], lA[:], mybir.ActivationFunctionType.Exp,
                    bias=mneg[:], accum_out=sA[:],
                )
                nc.scalar.activation(
                    sm[:, NSPLIT:OUT], lB[:], mybir.ActivationFunctionType.Exp,
                    bias=mneg[:], accum_out=sB[:],
                )
                nc.vector.tensor_add(stot[:], sA[:], sB[:])
                nc.vector.reciprocal(rec[:], stot[:])
                nc.vector.tensor_scalar_mul(sm[:], sm[:], rec[:])
                nc.gpsimd.dma_start(out[:], sm[:])

    nc.compile()
    return nc


def _get_nc(T):
    key = (T, REP, H_BF16, X_BF16, FUSED, SCAN, SIG3)
    if key not in _CACHE:
        _CACHE[key] = _build(T)
    return _CACHE[key]


def prep_inputs(x, w_ih, w_hh, b_ih, b_hh, w_dense, b_dense):
    import ml_dtypes
    xd = ml_dtypes.bfloat16 if X_BF16 else np.float32
    hd = ml_dtypes.bfloat16 if H_BF16 else np.float32
    T = x.shape[0]
    x = np.ascontiguousarray(x, dtype=np.float32)
    # xT[k, kt, t, b] = x[t, b, kt*128+k]
    xt_all = np.ascontiguousarray(
        x.reshape(T, B, KT, H).transpose(3, 2, 0, 1).astype(xd)
    )
    whhT = np.ascontiguousarray(
        w_hh.reshape(4, H, H)[PERM].transpose(2, 0, 1).astype(hd)
    )
    wihT = np.ascontiguousarray(
        w_ih.reshape(4, H, KT, H)[PERM].transpose(3, 2, 0, 1).astype(xd)
    )
    bias4 = (b_ih + b_hh).reshape(4, H)[PERM].astype(np.float32)
    # pre-scale the g gate by 2: tanh(x) = 2*sigmoid(2x) - 1
    whhT = whhT.copy(); wihT = wihT.copy()
    whhT[:, GPRE, :] = whhT[:, GPRE, :] * np.asarray(2.0, whhT.dtype)
    wihT[:, :, GPRE, :] = wihT[:, :, GPRE, :] * np.asarray(2.0, wihT.dtype)
    bias4[GPRE] *= 2.0
    bias4 = np.ascontiguousarray(bias4.astype(xd))
    # ind4[g, n] for n = s*(G4*BC) + gq*BC + b  -> 1.0 iff gq == g
    ind4 = np.zeros((G4, SPB * G4 * BC), dtype=xd)
    nidx = np.arange(SPB * G4 * BC)
    gq = (nidx // BC) % G4
    for g in range(G4):
        ind4[g, gq == g] = 1.0
    import ml_dtypes as _mld
    wdT = np.ascontiguousarray(w_dense.T.astype(_mld.bfloat16))
    bd = np.ascontiguousarray(b_dense.reshape(1, OUT).astype(_mld.bfloat16))

    in_maps = []
    for c in range(N_CORES):
        in_maps.append(
            {
                "xT": np.ascontiguousarray(xt_all[:, :, :, c * BC : (c + 1) * BC]),
                "whhT": whhT,
                "wihT": wihT,
                "bias4": bias4,
                "ind4": ind4,
                "wdT": wdT,
                "bd": bd,
            }
        )
    return in_maps


def kernel(x, w_ih, w_hh, b_ih, b_hh, w_dense, b_dense):
    x = np.asarray(x)
    T = x.shape[0]
    nc = _get_nc(T)
    in_maps = prep_inputs(
        np.asarray(x), np.asarray(w_ih), np.asarray(w_hh),
        np.asarray(b_ih), np.asarray(b_hh),
        np.asarray(w_dense), np.asarray(b_dense),
    )
    res = run_bass_kernel_spmd(nc, in_maps, list(range(N_CORES)))
    return np.concatenate(
        [res.results[c]["out"] for c in range(N_CORES)], axis=0
    ).astype(np.float32)



# revision 36
# speedup vs baseline: 1.2060x; 1.0003x over previous
"""Trainium2 Bass kernel for the LSTM+dense+softmax model.

Model (see reference): x[T=512, B=256, IN=256] -> LSTM(H=128) last hidden
-> dense(OUT=1000) -> softmax. Data-parallel over batch across 8 cores
(32 batch elements per core), weights replicated.

Layout: recurrent state is kept transposed [H=128 partitions, batch] so the
per-step W_hh matmuls, gate nonlinearities and cell update all run at full
partition width with no transposes. Gate pre-activations for 4 consecutive
steps share one PSUM bank: W_ih*x contributions (+bias) are accumulated
ahead of time, W_hh*h is added when the step arrives, and ScalarE applies
sigmoid directly out of PSUM. Both matmul paths run in bfloat16.

The recurrence is LATENCY-bound: 512 serial steps, each a cross-engine
dependency chain whose cost is per-instruction fixed overheads (ScalarE op
~300ns, DVE op ~170ns, PE SBUF fill ~180ns, ~35ns/semaphore hop), not
data width. Optimizations here, measured per step on hardware:
  - one 4-gate sigmoid (g pre-doubled; tanh(g)=2*sig(2g)-1 folded into the
    cell update) -- a split (i,f,g)+(o) sigmoid measured slower twice.
  - the ENTIRE cell update c=i*tanh(g)+f*c is ONE custom DVE instruction
    (LSTM_SCAN_ANT): a hand-built 3-uop ping-pong FSM over interleaved
    (even,odd) element pairs -- even elements compute i*(2sg-1) resetting
    the running sum, odd elements add f*c and emit c_new. Gate/cell values
    live interleaved in two ping-pong "sets" so every operand of every op
    stays a single affine access pattern. (The stock DSL has no segmented
    scan; the uop program is derived by patching a lowered scan template.)
  - x-projection matmuls + the bias matmul are dep-anchored (add_dep_helper
    on each step's sigmoid) so the simulation-guided scheduler spreads them
    into per-step PE-idle windows; emission-order/priority placement alone
    measured +280..560ns spikes on group-boundary steps.
  - dense-layer bias matmuls precede the h matmuls (no dependency on h),
    small x chunks (CH=8) + AHEAD=2 priming cut kernel head time.
Step ~1650ns: sig4(374) SCAN(240) tanh(321) hmul(191) 4x W_hh mm(268) +
sem hops. Measured slower/dead: sigmoid split, per-element DVE ops (3-op
cell), Pool offload, emission-order xproj placement, fp32 matmul paths.
"""

import numpy as np

import concourse.bacc as bacc
import concourse.mybir as mybir
import concourse.tile as tile
from concourse.bass_utils import run_bass_kernel_spmd

SEQ = 512
B = 256
IN = 256
H = 128
OUT = 1000
N_CORES = 8
BC = B // N_CORES  # 32 batch per core
KT = IN // H  # 2 k-tiles for the input projection
G4 = 4  # gate slot order: (i,f,g,o) in SCAN mode, (i,f,o,g) otherwise
SPB = 4  # steps per PSUM bank group (4*4*32 fp32 = one 2KB bank)
AHEAD = 2  # bank groups of x-projection lookahead
CH = 8  # timesteps per streamed x chunk (small first chunks cut startup)

F32 = mybir.dt.float32
BF16 = mybir.dt.bfloat16

import os as _os
H_BF16 = _os.environ.get("LSTM_H_BF16", "1") == "1"  # W_hh*h path in bf16
X_BF16 = _os.environ.get("LSTM_X_BF16", "1") == "1"  # W_ih*x (+bias) path in bf16
REP = int(_os.environ.get("LSTM_REP", "1"))  # timing amplification (bench only)
FUSED = _os.environ.get("LSTM_FUSED", "1") == "1"  # custom-DVE fused cell update

_CACHE = {}


# --- custom DVE op: fused LSTM cell products ------------------------------
# One instruction computing, over [P, 2, N] paged operands,
#   page 0: out = i * (2*sg - 1)   (= i * tanh(g~), sg = sigmoid(2*g~))
#   page 1: out = f * c
# replacing the stock MULTIPLY + MULTIPLY,SUBTRACT pair on the recurrence's
# serial critical path. Registered into concourse.dve_ops.OPS at import time
# (the documented per-NEFF extension point; the uop table is regenerated for
# every compile, no firmware change involved).
def _lstm_cell_ref(in0, in1, s0, s1, imm2):
    a = in0.reshape((in0.shape[0], 2, -1)).astype(np.float32)
    b = in1.reshape(a.shape).astype(np.float32)
    out = np.empty_like(a)
    out[:, 0, :] = b[:, 0, :] * (2.0 * a[:, 0, :] - 1.0)
    out[:, 1, :] = b[:, 1, :] * a[:, 1, :]
    return out.reshape(in0.shape)


def _register_lstm_cell_op():
    import re
    import concourse.dve_ops as dve_ops_mod
    from concourse.dve_ops import OPS, CUSTOM_DVE_SPECS, DveOp
    from concourse.dve_spec import Spec, Src0, Src1, One, SubIdx

    name = "LSTM_CELL_ANT"
    for op in OPS:
        if op.name == name:
            return op
    pg0 = SubIdx < One  # 1.0 on page 0, 0.0 on page 1
    spec = Spec(
        body=Src1 * (Src0 * (One + pg0) - pg0),
        reference=_lstm_cell_ref,
    )
    probe = DveOp(name, spec, subdim=True, uops_sha={})
    OPS.append(probe)
    dve_ops_mod._SUB_OPCODE_FOR_NAME[name] = (
        dve_ops_mod._CUSTOM_DVE_ROW_BASE + len(OPS) - 1
    )
    CUSTOM_DVE_SPECS[name] = spec
    shas = {}
    for ver in ("v3", "v4"):
        try:
            probe.compile(ver)
            shas[ver] = probe.uops_sha.get(ver, "")
        except ValueError as e:
            m = re.search(r'"([0-9a-f]{16})"', str(e))
            if not m:
                raise
            shas[ver] = m.group(1)
    final = DveOp(name, spec, subdim=True, uops_sha=shas)
    OPS[-1] = final
    from concourse.dve_table_gen import free_opcode_rows

    row = dve_ops_mod.get_dve_sub_opcode(name)
    assert row in free_opcode_rows("TRN2"), (name, row)
    return final


LSTM_CELL_OP = _register_lstm_cell_op() if FUSED else None

SCAN = _os.environ.get("LSTM_SCAN", "1") == "1"  # segmented-scan cell update
# split sigmoid (i,f,g)+(o): measured ~70ns/step SLOWER than one 4-gate
# sigmoid even with dep-anchored xproj — the second Act op's occupancy
# outweighs the earlier start. Keep off.
SIG3 = _os.environ.get("LSTM_SIG3", "0") == "1"


# --- custom DVE op: full cell update in ONE instruction -------------------
# Streams interleaved pairs (even, odd) = ((sg_b, c_b) via in0, (i_b, f_b)
# via in1) and emits at each odd position  c_new_b = i_b*(2*sg_b-1) + f_b*c_b.
# The stock DSL has no per-pair scan reset, so the uop program is hand-built:
# a 3-state ping-pong FSM (even resets the running sum to i*(2sg-1), odd adds
# f*c), derived by patching the lowered template of a plain scan spec.
def _lstm_scan_ref(in0, in1, s0, s1, imm2):
    P = in0.shape[0]
    a = in0.reshape(P, -1, 2).astype(np.float32)  # sg, c
    b = np.asarray(in1).reshape(P, -1, 2).astype(np.float32)  # i, f
    c0 = s0 if isinstance(s0, float) else s0.reshape(P, 1)
    it = b[:, :, 0] * (c0 * a[:, :, 0] - 1.0)
    fc = b[:, :, 1] * a[:, :, 1]
    out = np.empty_like(a)
    out[:, :, 0] = it
    out[:, :, 1] = it + fc
    return out.reshape(in0.shape)


def _register_lstm_scan_op():
    import copy
    import concourse.dve_ops as dve_ops_mod
    from concourse.dve_ops import OPS, CUSTOM_DVE_SPECS, DveOp
    from concourse.dve_spec import Spec, Src0, Src1, One, C0, scan, lower, AluOp
    from concourse.dve_uop import DveOpSpec, Trigger
    from dataclasses import dataclass

    name = "LSTM_SCAN_ANT"
    for op in OPS:
        if op.name == name:
            return op

    # template: x = (Src0*C0 - 1)*Src1 ; S = scan(ADD, x)
    x = ((Src0 * C0) - One) * Src1
    spec = Spec(body=scan(AluOp.ADD, x), reference=_lstm_scan_ref)

    @dataclass(frozen=True)
    class _PatchedDveOp(DveOp):
        programs: dict = None

        def compile(self, ver):
            return self.programs[ver]

    programs = {}
    shas = {}
    OPS.append(None)  # reserve the row before computing opcode
    row_idx = dve_ops_mod._CUSTOM_DVE_ROW_BASE + len(OPS) - 1
    dve_ops_mod._SUB_OPCODE_FOR_NAME[name] = row_idx
    CUSTOM_DVE_SPECS[name] = spec
    for ver in ("v3", "v4"):
        uops = lower(spec, ver=ver)
        assert len(uops) == 2, len(uops)
        steady = uops[1]
        # identify blocks: blk_mul2 (MUL Src0*C0), blk_sub (-1), blk_scan (ADD CURR)
        blk_mul2 = blk_sub = blk_scan = None
        for i, dp in enumerate(steady.datapath_config):
            if dp.op == AluOp.MULTIPLY and blk_mul2 is None:
                blk_mul2 = i
            elif dp.op == AluOp.SUBTRACT and blk_sub is None:
                blk_sub = i
            elif dp.op == AluOp.ADD and "CURR" in dp.alu_src0.name:
                blk_scan = i
        assert None not in (blk_mul2, blk_sub, blk_scan), (
            blk_mul2, blk_sub, blk_scan)

        def mk_even(nxt):
            u = copy.deepcopy(steady)
            dp = u.datapath_config[blk_scan]
            dp.op = AluOp.BYPASS
            dp.alu_src0 = dp.alu_src1  # pass x through; resets running sum
            u.trigger = (Trigger.SRC_TENSOR_DONE, Trigger.COUNT, Trigger.NONE)
            u.next_uop = (0, nxt, 0)
            u.repeat_count = 1
            return u

        def mk_odd(nxt):
            u = copy.deepcopy(steady)
            dpm = u.datapath_config[blk_mul2]
            dpm.op = AluOp.BYPASS  # x1 = Src0 (skip *C0)
            dps = u.datapath_config[blk_sub]
            dps.op = AluOp.BYPASS
            dps.alu_src1 = dps.alu_src0  # x2 = x1 (skip -1)
            u.trigger = (Trigger.SRC_TENSOR_DONE, Trigger.COUNT, Trigger.NONE)
            u.next_uop = (0, nxt, 0)
            u.repeat_count = 1
            return u

        prog = [mk_even(1), mk_odd(2), mk_even(1)]
        dos = DveOpSpec(name=name, opcode=row_idx, uops=prog, rd1_en=True)
        programs[ver] = dos
        shas[ver] = dos.sha(ver)

    final = _PatchedDveOp(name, spec, subdim=False, uops_sha=shas,
                          programs=programs)
    OPS[-1] = final
    from concourse.dve_table_gen import free_opcode_rows

    assert row_idx in free_opcode_rows("TRN2"), (name, row_idx)
    return final


LSTM_SCAN_OP = _register_lstm_scan_op() if SCAN else None

# torch gate block (i,f,g,o) -> our slot; GPRE = slot of the pre-doubled g
if SCAN:
    PERM, GPRE = [0, 1, 2, 3], 2
else:
    PERM, GPRE = [0, 1, 3, 2], 3


def _build(T):
    ngrp = T // SPB
    ch = min(CH, T)
    HD = BF16 if H_BF16 else F32
    XD = BF16 if X_BF16 else F32
    nc = bacc.Bacc("TRN2", target_bir_lowering=False, debug=False)

    xT = nc.declare_dram_parameter("xT", [H, KT, T, BC], XD, isOutput=False)
    whhT = nc.declare_dram_parameter("whhT", [H, G4, H], HD, isOutput=False)
    wihT = nc.declare_dram_parameter("wihT", [H, KT, G4, H], XD, isOutput=False)
    bias4 = nc.declare_dram_parameter("bias4", [G4, H], XD, isOutput=False)
    ind4 = nc.declare_dram_parameter("ind4", [G4, SPB * G4 * BC], XD, isOutput=False)
    wdT = nc.declare_dram_parameter("wdT", [H, OUT], BF16, isOutput=False)
    bd = nc.declare_dram_parameter("bd", [1, OUT], BF16, isOutput=False)
    out = nc.declare_dram_parameter("out", [BC, OUT], F32, isOutput=True)

    NSPLIT = 512  # dense tail: first PSUM bank columns
    NREST = OUT - NSPLIT

    with tile.TileContext(nc) as tc:
        with (
            tc.tile_pool(name="const", bufs=1) as constp,
            tc.tile_pool(name="xs", bufs=6) as xpool,
            tc.tile_pool(name="state", bufs=1) as state,
            tc.tile_pool(name="work", bufs=3) as work,
        ):
            whh_s = constp.tile([H, G4, H], HD)
            wih_s = constp.tile([H, KT, G4, H], XD)
            bias_s = constp.tile([G4, H], XD)
            ind_s = constp.tile([G4, SPB * G4 * BC], XD)
            wd_s = constp.tile([H, OUT], BF16)
            bd_s = constp.tile([1, OUT], BF16)
            ones1 = constp.tile([1, BC], BF16)
            nc.gpsimd.dma_start(whh_s[:], whhT[:])
            nc.gpsimd.dma_start(wih_s[:], wihT[:])
            nc.gpsimd.dma_start(bias_s[:], bias4[:])
            nc.gpsimd.dma_start(ind_s[:], ind4[:])
            nc.vector.memset(ones1[:], 1.0)

            # persistent state: h transposed [H, BC].
            # W = [sig(i) sig(f) sig(o) sig(2g) | c]: the sigmoid of all 4
            # (pre-scaled) gates lands in W[:,0:128] right next to the cell
            # state c in W[:,128:160], so [i|f] (x) [sig2g|c] is one
            # contiguous 64-wide multiply. tanh(g) = 2*sig(2g)-1 is folded
            # into the cell update (g weights are pre-doubled on the host).
            # (A 2-op cell update via a duplicated-sigmoid scatter measured
            # dead even on HW: the saved DVE op's fixed cost reappears in
            # the 2x-wider activation. This layout is the local optimum.)
            hT = state.tile([H, BC], HD)
            hT32 = state.tile([H, BC], BF16)
            # SCAN mode: two ping-pong cell sets, each 4 blocks of 2*BC
            # (i, f, sg/c, o) with gate values at even offsets and the cell
            # state c at the odd offsets of the sg block. Otherwise the flat
            # [i f o sg | c] layout.
            W = state.tile([H, 16 * BC] if SCAN else [H, 5 * BC], F32)

            nchunk = (T + ch - 1) // ch
            xtiles = [None] * nchunk

            def ensure_chunk(ci):
                if xtiles[ci] is None:
                    xt = xpool.tile([H, KT, ch, BC], XD)
                    nc.gpsimd.dma_start(
                        xt[:], xT[:, :, ci * ch : (ci + 1) * ch, :]
                    )
                    xtiles[ci] = xt

            for _rep in range(REP):
              if True:
                xtiles = [None] * nchunk
                nc.vector.memset(hT[:], 0.0)
                nc.vector.memset(W[:], 0.0)
                psump_cm = tc.tile_pool(name=f"psum{_rep}", bufs=AHEAD + 2, space="PSUM")
                psump = psump_cm.__enter__()
                pstiles = [None] * ngrp

                from concourse.tile import add_dep_helper

                def alloc_group(g, dep=None):
                    # allocate the PSUM bank for group g and seed it with the
                    # bias: the ONE start=True matmul covering the whole bank
                    # (start=True clears has_written bank-wide, so it must be
                    # the single first writer; everything after accumulates)
                    t0 = g * SPB
                    ensure_chunk(t0 // ch)
                    ps = psump.tile([H, SPB, G4, BC], F32)
                    pstiles[g] = ps
                    mm = nc.tensor.matmul(
                        ps[:].rearrange("p a g b -> p (a g b)"),
                        bias_s[:],
                        ind_s[:],
                        start=True,
                        stop=False,
                        skip_group_check=True,
                    )
                    if dep is not None:
                        add_dep_helper(mm.ins, dep.ins, reason="xproj window anchor")

                def emit_xproj_mms(g, pairs, dep=None):
                    # accumulate W_ih*x contributions (gi, kt) for group g
                    t0 = g * SPB
                    ci = t0 // ch
                    xt = xtiles[ci]
                    s0 = t0 - ci * ch
                    ps = pstiles[g]
                    for gi, kt in pairs:
                        mm = nc.tensor.matmul(
                            ps[:, :, gi, :],
                            wih_s[:, kt, gi, :],
                            xt[:, kt, s0 : s0 + SPB, :],
                            start=False,
                            stop=False,
                            skip_group_check=True,
                        )
                        if dep is not None:
                            add_dep_helper(mm.ins, dep.ins, reason="xproj window anchor")

                ALL_PAIRS = [(gi, kt) for gi in range(G4) for kt in range(KT)]

                for g in range(min(AHEAD, ngrp)):
                    alloc_group(g)
                    emit_xproj_mms(g, ALL_PAIRS)

                # next group's xproj is spread across this group's four
                # steps, each sub-batch dep-anchored on that step's sigmoid
                # so the scheduler places it in the step's PE-idle window
                # (one 9-op batch at a group boundary overflows the window)
                XPIECES = [[], ALL_PAIRS[0:3], ALL_PAIRS[3:6], ALL_PAIRS[6:8]]
                for g in range(ngrp):
                    ps = pstiles[g]
                    for s in range(SPB):
                        t_glob = g * SPB + s
                        # W_hh * h into the gate bank (critical path).
                        for gi in ((0, 1, 2, 3) if SCAN else (3, 0, 1, 2)):
                            nc.tensor.matmul(
                                ps[:, s, gi, :],
                                whh_s[:, gi, :],
                                hT[:],
                                start=False,
                                stop=(gi == (3 if SCAN else 2)),
                                skip_group_check=True,
                            )
                        tct = work.tile([H, BC], F32)
                        ht_dst = hT32 if t_glob == T - 1 else hT
                        if SCAN:
                            Xb = 0 if (t_glob % 2 == 0) else 8 * BC
                            Yb = 8 * BC - Xb
                            if SIG3:
                                # sig(i,f,g) only waits on the first 3 W_hh
                                # matmuls; sig(o) runs behind it on ScalarE,
                                # hidden under the DVE cell op (o is not
                                # needed until the h-mul)
                                sig_inst = nc.scalar.activation(
                                    W[:, Xb : Xb + 6 * BC].rearrange(
                                        "p (g b two) -> p g b two", g=3, two=2
                                    )[:, :, :, 0],
                                    ps[:, s, 0:3, :],
                                    mybir.ActivationFunctionType.Sigmoid,
                                )
                                nc.scalar.activation(
                                    W[:, Xb + 6 * BC : Xb + 8 * BC].rearrange(
                                        "p (b two) -> p b two", two=2
                                    )[:, :, 0],
                                    ps[:, s, 3, :],
                                    mybir.ActivationFunctionType.Sigmoid,
                                )
                            else:
                                # sigmoid of all 4 gates -> even offsets of X
                                sig_inst = nc.scalar.activation(
                                    W[:, Xb : Xb + 8 * BC].rearrange(
                                        "p (g b two) -> p g b two", g=4, two=2
                                    )[:, :, :, 0],
                                    ps[:, s, :, :],
                                    mybir.ActivationFunctionType.Sigmoid,
                                )
                            # whole cell update in one segmented-scan DVE op:
                            # set Y's sg/c block gets [junk | c_new] pairs
                            in1 = W[:, Xb : Xb + 4 * BC].rearrange(
                                "p (j b two) -> p j b two", j=2, two=2
                            )[:, :, :, 0].rearrange("p j b -> p b j")
                            nc.vector._custom_dve(
                                LSTM_SCAN_OP,
                                out=W[:, Yb + 4 * BC : Yb + 6 * BC],
                                in0=W[:, Xb + 4 * BC : Xb + 6 * BC],
                                in1=in1,
                                s0=2.0,
                            )
                            tanh_inst = nc.scalar.activation(
                                tct[:],
                                W[:, Yb + 4 * BC : Yb + 6 * BC].rearrange(
                                    "p (b two) -> p b two", two=2
                                )[:, :, 1],
                                mybir.ActivationFunctionType.Tanh,
                            )
                            nc.vector.tensor_mul(
                                ht_dst[:],
                                W[:, Xb + 6 * BC : Xb + 8 * BC].rearrange(
                                    "p (b two) -> p b two", two=2
                                )[:, :, 0],
                                tct[:],
                            )
                        else:
                            prod = work.tile([H, 2 * BC], F32)
                            # sigmoid of all 4 gates (g pre-scaled by 2)
                            sig_inst = nc.scalar.activation(
                                W[:, 0 : 4 * BC].rearrange("p (g b) -> p g b", g=4),
                                ps[:, s, :, :],
                                mybir.ActivationFunctionType.Sigmoid,
                            )
                            if FUSED:
                                # fused DVE op: prod = [i*(2*sig2g-1) | f*c]
                                nc.vector._custom_dve(
                                    LSTM_CELL_OP,
                                    out=prod[:].rearrange("p (s b) -> p s b", s=2),
                                    in0=W[:, 3 * BC : 5 * BC].rearrange(
                                        "p (s b) -> p s b", s=2
                                    ),
                                    in1=W[:, 0 : 2 * BC].rearrange(
                                        "p (s b) -> p s b", s=2
                                    ),
                                )
                                # c = i*tanh(g) + f*c
                                nc.vector.tensor_add(
                                    W[:, 4 * BC : 5 * BC],
                                    prod[:, 0:BC],
                                    prod[:, BC : 2 * BC],
                                )
                            else:
                                # prod = [i*sig2g | f*c]
                                nc.vector.tensor_mul(
                                    prod[:], W[:, 0 : 2 * BC], W[:, 3 * BC : 5 * BC]
                                )
                                # c = 2*prod0 - i + prod1
                                nc.vector.scalar_tensor_tensor(
                                    tct[:], prod[:, 0:BC], 2.0, W[:, 0:BC],
                                    op0=mybir.AluOpType.mult,
                                    op1=mybir.AluOpType.subtract,
                                )
                                nc.vector.tensor_add(
                                    W[:, 4 * BC : 5 * BC], tct[:],
                                    prod[:, BC : 2 * BC],
                                )
                            tanh_inst = nc.scalar.activation(
                                tct[:],
                                W[:, 4 * BC : 5 * BC],
                                mybir.ActivationFunctionType.Tanh,
                            )
                            nc.vector.tensor_mul(
                                ht_dst[:], W[:, 2 * BC : 3 * BC], tct[:]
                            )
                        if g + AHEAD < ngrp:
                            if s == 0:
                                alloc_group(g + AHEAD, dep=sig_inst)
                            emit_xproj_mms(g + AHEAD, XPIECES[s][:2], dep=sig_inst)
                            # last piece anchored on tanh: keeps the PE pipe
                            # streaming until ~150ns before the W_hh matmuls,
                            # shrinking their cold SBUF-access fill
                            emit_xproj_mms(g + AHEAD, XPIECES[s][2:], dep=tanh_inst)
                    pstiles[g] = None

                psump_cm.__exit__(None, None, None)
            # dense + softmax tail (weight DMAs were deferred so the
            # recurrence's x chunks got the DGE queue first)
            nc.gpsimd.dma_start(wd_s[:], wdT[:])
            nc.gpsimd.dma_start(bd_s[:], bd[:])
            with tc.tile_pool(name="psd", bufs=2, space="PSUM") as psumd:
                # preload the exp activation table during the dense matmuls
                # (otherwise the first EXP pays the 1.3us table load inline)
                warm_exp = work.tile([1, BC], F32)
                wx = nc.scalar.activation(
                    warm_exp[:], ones1[:], mybir.ActivationFunctionType.Exp
                )
                add_dep_helper(wx.ins, tanh_inst.ins, reason="exp table preload")
                lA = psumd.tile([BC, NSPLIT], F32)
                lB = psumd.tile([BC, NREST], F32)
                # bias matmuls first: they don't depend on the last hidden
                # state, so they run during the final step's elementwise ops
                nc.tensor.matmul(
                    lA[:], ones1[:], bd_s[:, 0:NSPLIT], start=True, stop=False,
                    skip_group_check=True,
                )
                nc.tensor.matmul(
                    lB[:], ones1[:], bd_s[:, NSPLIT:OUT], start=True, stop=False,
                    skip_group_check=True,
                )
                nc.tensor.matmul(
                    lA[:], hT32[:], wd_s[:, 0:NSPLIT], start=False, stop=True,
                    skip_group_check=True,
                )
                nc.tensor.matmul(
                    lB[:], hT32[:], wd_s[:, NSPLIT:OUT], start=False, stop=True,
                    skip_group_check=True,
                )
                mA = work.tile([BC, 1], F32)
                mB = work.tile([BC, 1], F32)
                mneg = work.tile([BC, 1], F32)
                sA = work.tile([BC, 1], F32)
                sB = work.tile([BC, 1], F32)
                stot = work.tile([BC, 1], F32)
                rec = work.tile([BC, 1], F32)
                sm = work.tile([BC, OUT], F32)
                nc.vector.reduce_max(mA[:], lA[:], axis=mybir.AxisListType.X)
                nc.vector.reduce_max(mB[:], lB[:], axis=mybir.AxisListType.X)
                nc.vector.tensor_max(mA[:], mA[:], mB[:])
                nc.vector.tensor_scalar_mul(mneg[:], mA[:], -1.0)
                nc.scalar.activation(
                    sm[:, 0:NSPLIT], lA[:], mybir.ActivationFunctionType.Exp,
                    bias=mneg[:], accum_out=sA[:],
                )
                nc.scalar.activation(
                    sm[:, NSPLIT:OUT], lB[:], mybir.ActivationFunctionType.Exp,
                    bias=mneg[:], accum_out=sB[:],
                )
                nc.vector.tensor_add(stot[:], sA[:], sB[:])
                nc.vector.reciprocal(rec[:], stot[:])
                nc.vector.tensor_scalar_mul(sm[:], sm[:], rec[:])
                nc.gpsimd.dma_start(out[:], sm[:])

    nc.compile()
    return nc


def _get_nc(T):
    key = (T, REP, H_BF16, X_BF16, FUSED, SCAN, SIG3)
    if key not in _CACHE:
        _CACHE[key] = _build(T)
    return _CACHE[key]


def prep_inputs(x, w_ih, w_hh, b_ih, b_hh, w_dense, b_dense):
    import ml_dtypes
    xd = ml_dtypes.bfloat16 if X_BF16 else np.float32
    hd = ml_dtypes.bfloat16 if H_BF16 else np.float32
    T = x.shape[0]
    x = np.ascontiguousarray(x, dtype=np.float32)
    # xT[k, kt, t, b] = x[t, b, kt*128+k]
    xt_all = np.ascontiguousarray(
        x.reshape(T, B, KT, H).transpose(3, 2, 0, 1).astype(xd)
    )
    whhT = np.ascontiguousarray(
        w_hh.reshape(4, H, H)[PERM].transpose(2, 0, 1).astype(hd)
    )
    wihT = np.ascontiguousarray(
        w_ih.reshape(4, H, KT, H)[PERM].transpose(3, 2, 0, 1).astype(xd)
    )
    bias4 = (b_ih + b_hh).reshape(4, H)[PERM].astype(np.float32)
    # pre-scale the g gate by 2: tanh(x) = 2*sigmoid(2x) - 1
    whhT = whhT.copy(); wihT = wihT.copy()
    whhT[:, GPRE, :] = whhT[:, GPRE, :] * np.asarray(2.0, whhT.dtype)
    wihT[:, :, GPRE, :] = wihT[:, :, GPRE, :] * np.asarray(2.0, wihT.dtype)
    bias4[GPRE] *= 2.0
    bias4 = np.ascontiguousarray(bias4.astype(xd))
    # ind4[g, n] for n = s*(G4*BC) + gq*BC + b  -> 1.0 iff gq == g
    ind4 = np.zeros((G4, SPB * G4 * BC), dtype=xd)
    nidx = np.arange(SPB * G4 * BC)
    gq = (nidx // BC) % G4
    for g in range(G4):
        ind4[g, gq == g] = 1.0
    import ml_dtypes as _mld
    wdT = np.ascontiguousarray(w_dense.T.astype(_mld.bfloat16))
    bd = np.ascontiguousarray(b_dense.reshape(1, OUT).astype(_mld.bfloat16))

    in_maps = []
    for c in range(N_CORES):
        in_maps.append(
            {
                "xT": np.ascontiguousarray(xt_all[:, :, :, c * BC : (c + 1) * BC]),
                "whhT": whhT,
                "wihT": wihT,
                "bias4": bias4,
                "ind4": ind4,
                "wdT": wdT,
                "bd": bd,
            }
        )
    return in_maps


def kernel(x, w_ih, w_hh, b_ih, b_hh, w_dense, b_dense):
    x = np.asarray(x)
    T = x.shape[0]
    nc = _get_nc(T)
    in_maps = prep_inputs(
        np.asarray(x), np.asarray(w_ih), np.asarray(w_hh),
        np.asarray(b_ih), np.asarray(b_hh),
        np.asarray(w_dense), np.asarray(b_dense),
    )
    res = run_bass_kernel_spmd(nc, in_maps, list(range(N_CORES)))
    return np.concatenate(
        [res.results[c]["out"] for c in range(N_CORES)], axis=0
    ).astype(np.float32)

